# revision 19
# baseline (speedup 1.0000x reference)
"""Trainium2 Bass kernel for a post-LN multi-head-attention block.

Reference computation (B=4, S=2048, D=1024, 16 heads x 64):
    q,k,v = x @ W{q,k,v}.T ; attn = softmax(q k^T/8 + mask) ; o = attn v
    out = LayerNorm(query + (o @ Wo.T)) * gamma + beta

Sharding: 8 cores = 4 batches x 2 query-halves (1024 query rows per core).
Each core computes all 16 heads for its query rows against the full
(mask-compacted) key set of its batch.  No collectives.

Key implementation choices:
  - keys with mask==0 contribute exactly 0 attention weight, so the host
    compacts key/value to the unmasked subset, padded to a multiple of 128
    (padding biased -1e30 so exp -> 0).
  - Q/K/V/O projections and attnV run in fp8e4m3 with DoubleRow perf
    mode (2 contraction rows per PE pass = 2x f32r/bf16 matmul
    throughput).  The attention output is only ~5% of the residual
    magnitude entering LayerNorm, so fp8 noise dilutes ~20x in the final
    output (measured end-to-end rel err ~9e-3 vs the 2e-2 budget).
  - scores keep fp16 operands; exp bias includes -4.5 so fp8
    e2 = exp(qk/8 - 4.5) stays below e4m3 max 240.
  - scores are computed transposed, scoresT[k, q], so softmax's
    k-reduction becomes a matmul reduction: V is augmented with a
    ones-column and attnV produces [out^T ; rowsum] in one PSUM group.
    Even heads use a [V | 1] augmentation (accumulator rows 0-64), odd
    heads a [1 | V] augmentation placed at PSUM partitions 63-127, so
    the normalized output lands directly in aoT partitions 64-127
    without a partition-shift DMA.
  - ScalarE (softmax exp) is the bottleneck engine (~137us of exp at
    1 elem/cycle/lane); everything else is scheduled around keeping it
    saturated: the prologue only computes the two projection chunks the
    first exp needs (their DMAs spread over four DGE queues), all other
    projections are deadline-ordered fills inside the kt loops, each
    iteration's first scores matmul is pre-issued at the tail of the
    previous iteration, and LayerNorm rstd uses Ln+Exp (one activation
    table set with the softmax Exp -> no table reloads).
  - loop nest is qc-outer / head-pair-inner: projections fill the
    TensorE during qc0's attention, and the output projection +
    LayerNorm for qc0 overlaps qc1's attention; the qc1 epilogue
    applies LayerNorm on ScalarE (idle after the last exp).
  - gamma/beta are ones/zeros in this problem; make_in_maps verifies at
    runtime and compiles the gamma/beta application out (falling back to
    a gpsimd apply if they were ever nontrivial).
"""

import numpy as np
import ml_dtypes

import concourse.bacc as bacc
import concourse.tile as tile
import concourse.bass as bass
from concourse import mybir
from concourse.bass_utils import run_bass_kernel_spmd

DMODEL = 1024
NHEAD = 16
HD = 64
B = 4
S = 2048
NCORES = 8
SQ = 1024          # query rows per core
P = 128
F32 = mybir.dt.float32
F32R = mybir.dt.float32r
F16 = mybir.dt.float16
BF16 = mybir.dt.bfloat16
F8 = mybir.dt.float8e4
DR = mybir.MatmulPerfMode.DoubleRow
NPF8 = ml_dtypes.float8_e4m3
ET = DMODEL // P   # 8 e-tiles (feature tiles)
DTL = DMODEL // P  # 8 d-tiles (contraction tiles)
NQC = SQ // 512    # 2 query chunks of 512
NEC = DMODEL // 512  # 2 feature chunks of 512
NPAIR = NHEAD // 2   # 8 head pairs; pair j = heads (2j, 2j+1) in e-tile j
AF = mybir.ActivationFunctionType


def _balanced_chunks(total, maxw=512):
    """Split `total` (a multiple of 128) into maximal chunks <= maxw,
    widest first: 512B-contiguous DMA runs dodge the sub-512B 2x
    latency penalty."""
    out, lo = [], 0
    while lo < total:
        w = min(maxw, total - lo)
        out.append((lo, lo + w))
        lo += w
    return out


def _build(LPAD, ln_trivial=True, do_compile=True, reps=1, phases=5):
    KT = LPAD // P
    KCH = _balanced_chunks(LPAD)
    QCH = _balanced_chunks(SQ)
    nc = bacc.Bacc("TRN2", target_bir_lowering=False, debug=False,
                   num_devices=NCORES)

    qT = nc.declare_dram_parameter("qT", [DMODEL, SQ], F8, isOutput=False)
    kT = nc.declare_dram_parameter("kT", [DMODEL, LPAD], F8, isOutput=False)
    vT = nc.declare_dram_parameter("vT", [DMODEL, LPAD], F8, isOutput=False)
    resid = nc.declare_dram_parameter("resid", [SQ, DMODEL], BF16,
                                      isOutput=False)
    wqT = nc.declare_dram_parameter("wqT", [DMODEL, DMODEL], F8, isOutput=False)
    wkT = nc.declare_dram_parameter("wkT", [DMODEL, DMODEL], F8, isOutput=False)
    wvT = nc.declare_dram_parameter("wvT", [DMODEL, DMODEL], F8, isOutput=False)
    woT = nc.declare_dram_parameter("woT", [DMODEL, DMODEL], F8, isOutput=False)
    maskb = nc.declare_dram_parameter("maskb", [P, KT], F32, isOutput=False)
    gamma = nc.declare_dram_parameter("gamma", [DMODEL], F32, isOutput=False)
    beta = nc.declare_dram_parameter("beta", [DMODEL], F32, isOutput=False)
    out = nc.declare_dram_parameter("out", [SQ, DMODEL], F32, isOutput=True)

    def dram3(ap):
        # (o*P, width) DRAM tensor viewed as [p, o, width]
        return ap.rearrange("(o p) w -> p o w", p=P)

    with tile.TileContext(nc) as tc:
        with (
            tc.tile_pool(name="keep", bufs=1) as keep,      # long-lived SBUF
            tc.tile_pool(name="dpool", bufs=2) as dpool,
            tc.tile_pool(name="pproj", bufs=2, space="PSUM") as pproj,
            tc.tile_pool(name="pscore", bufs=2, space="PSUM") as pscore,
            tc.tile_pool(name="pattn", bufs=1, space="PSUM") as pattn,
        ):
            for _rep in range(reps):
                self_body(nc, tc, keep, dpool, pproj, pscore, pattn, phases,
                          ln_trivial, LPAD, KT, KCH, QCH,
                          qT, kT, vT, resid, wqT, wkT, wvT, woT,
                          maskb, gamma, beta, out, dram3)
    if do_compile:
        nc.compile()
    return nc


def self_body(nc, tc, keep, dpool, pproj, pscore, pattn, phases, ln_trivial,
              LPAD, KT, KCH, QCH,
              qT, kT, vT, resid, wqT, wkT, wvT, woT, maskb, gamma, beta,
              out, dram3):
    NKTP = KT // 2            # full DoubleRow kt-pairs in attnV
    ODD = KT % 2
    NKC = len(KCH)
    NQCH = len(QCH)

    # ---- long-lived tensors ----
    vaug = keep.tile([P, KT, NHEAD, HD + 1], F8)     # [k | head | V,1]
    aoT = keep.tile([P, ET, SQ], F8)                 # attn out^T (d' on part)
    maskb_sb = keep.tile([P, KT], F32)
    nc.sync.dma_start(out=maskb_sb, in_=maskb.ap())
    ones_sb = keep.tile([65, 128], F32R)
    nc.vector.memset(ones_sb[64:65, :].bitcast(F32), 1.0)
    nc.vector.memset(ones_sb[0:1, :].bitcast(F32), 1.0)
    nc.vector.memset(vaug[:, :, :, HD:HD + 1], 1.0)
    kts = [keep.tile([P, LPAD], F16, name=f"kts{j}")
           for j in range(NPAIR)]
    qts = [keep.tile([P, SQ], F16, name=f"qts{j}")
           for j in range(NPAIR)]

    vT3 = dram3(vT.ap())
    wvT3 = dram3(wvT.ap())
    kT3 = dram3(kT.ap())
    qT3 = dram3(qT.ap())
    wkT3 = dram3(wkT.ap())
    wqT3 = dram3(wqT.ap())
    wk_sb = keep.tile([P, DTL, DMODEL], F8)
    wq_sb = keep.tile([P, DTL, DMODEL], F8)
    wv_sb = keep.tile([P, DTL, DMODEL], F8)
    kin = keep.tile([P, DTL, LPAD], F8)
    qin = keep.tile([P, DTL, SQ], F8)
    vin = keep.tile([P, DTL, LPAD], F8)
    K0 = KCH[0][1]
    Q0 = QCH[0][1]
    PKC0 = K0
    W0 = 512               # weight slice covering pairs 0-3
    # DMA bandwidth is one shared resource, so queue parallelism cannot
    # help the ramp -- ordering can.  Everything goes on the SP queue in
    # strict first-use order: the four transfers the first exp needs
    # (pair-0 weight columns, first key/query chunks) lead the stream.
    # DMA bandwidth is one shared resource and sub-512B-contiguous
    # slices pay a 2x latency penalty, so the stream is ordered by first
    # use with slice widths chosen around that penalty: 128-col weight
    # slices unblock pair 0/1 early (narrow but short), key/query chunks
    # stay >=512B, and the wide remainders stream at full rate.
    W1 = min(2 * P, DMODEL)
    nc.sync.dma_start(out=wk_sb[:, :, 0:P], in_=wkT3[:, :, 0:P])
    nc.sync.dma_start(out=kin[:, :, 0:K0], in_=kT3[:, :, 0:K0])
    nc.sync.dma_start(out=wq_sb[:, :, 0:P], in_=wqT3[:, :, 0:P])
    nc.sync.dma_start(out=qin[:, :, 0:Q0], in_=qT3[:, :, 0:Q0])
    if K0 < LPAD:
        nc.sync.dma_start(out=kin[:, :, K0:LPAD], in_=kT3[:, :, K0:LPAD])
    nc.sync.dma_start(out=wk_sb[:, :, P:W1], in_=wkT3[:, :, P:W1])
    nc.sync.dma_start(out=wq_sb[:, :, P:W1], in_=wqT3[:, :, P:W1])
    nc.sync.dma_start(out=wv_sb[:, :, 0:P], in_=wvT3[:, :, 0:P])
    nc.sync.dma_start(out=vin[:, :, 0:K0], in_=vT3[:, :, 0:K0])
    if K0 < LPAD:
        nc.sync.dma_start(out=vin[:, :, K0:LPAD], in_=vT3[:, :, K0:LPAD])
    nc.sync.dma_start(out=wv_sb[:, :, P:W1], in_=wvT3[:, :, P:W1])
    nc.sync.dma_start(out=wk_sb[:, :, W1:DMODEL], in_=wkT3[:, :, W1:DMODEL])
    nc.sync.dma_start(out=wq_sb[:, :, W1:DMODEL], in_=wqT3[:, :, W1:DMODEL])
    nc.sync.dma_start(out=wv_sb[:, :, W1:DMODEL], in_=wvT3[:, :, W1:DMODEL])
    for (lo, hi) in QCH[1:]:
        nc.sync.dma_start(out=qin[:, :, lo:hi], in_=qT3[:, :, lo:hi])

    def kth(j, lo, hi):
        """K^T projection thunk: kts[j][:, lo:hi] (fp8 DoubleRow, drained
        straight to fp16)."""
        def f(j=j, lo=lo, hi=hi):
            w = hi - lo
            ps = pproj.tile([P, 512], F32, tag="pp")
            for dt in range(DTL // 2):
                nc.tensor.matmul(
                    ps[:, :w],
                    lhsT=wk_sb[:, 2 * dt:2 * dt + 2, j * P:(j + 1) * P],
                    rhs=kin[:, 2 * dt:2 * dt + 2, lo:hi],
                    start=(dt == 0), stop=(dt == DTL // 2 - 1),
                    perf_mode=DR)
            nc.vector.tensor_copy(out=kts[j][:, lo:hi], in_=ps[:, :w])
        return f

    def qth(j, c):
        """Q^T projection thunk: qts[j][:, QCH[c]]."""
        def f(j=j, c=c):
            lo, hi = QCH[c]
            w = hi - lo
            ps = pproj.tile([P, 512], F32, tag="pp")
            for dt in range(DTL // 2):
                nc.tensor.matmul(
                    ps[:, :w],
                    lhsT=wq_sb[:, 2 * dt:2 * dt + 2, j * P:(j + 1) * P],
                    rhs=qin[:, 2 * dt:2 * dt + 2, lo:hi],
                    start=(dt == 0), stop=(dt == DTL // 2 - 1),
                    perf_mode=DR)
            nc.vector.tensor_copy(out=qts[j][:, lo:hi], in_=ps[:, :w])
        return f

    # V-projection per pair: kt-blocks grouped ~3 per thunk so a pair's
    # V work fits one fill slot
    _vstep = min((KT + 2) // 3, 4)
    VGRP = [(a, min(a + _vstep, KT)) for a in range(0, KT, _vstep)]
    NVG = len(VGRP)

    def vth(j, g):
        def f(j=j, g=g):
            g0, g1 = VGRP[g]
            gl = g1 - g0
            ps = pproj.tile([P, 512], F32, tag="pp")
            for i in range(gl):
                for dt in range(DTL // 2):
                    nc.tensor.matmul(
                        ps[:, i * P:(i + 1) * P],
                        lhsT=vin[:, 2 * dt:2 * dt + 2,
                                 (g0 + i) * P:(g0 + i + 1) * P],
                        rhs=wv_sb[:, 2 * dt:2 * dt + 2,
                                  j * P:(j + 1) * P],
                        start=(dt == 0), stop=(dt == DTL // 2 - 1),
                        perf_mode=DR)
            nc.vector.tensor_copy(
                out=vaug[:, g0:g1, 2 * j:2 * j + 2, 0:HD],
                in_=ps[:, 0:gl * P].rearrange(
                    "p (k h x) -> p k h x", h=2, x=HD))
        return f

    # ---- phase E pieces (output proj + residual + layernorm);
    # wo DMA is issued after the prologue so it doesn't delay vin ----
    wo_sb = keep.tile([P, DTL, DMODEL], F8)
    if not ln_trivial:
        gamma_sb = keep.tile([P, DMODEL], F32)
        beta_sb = keep.tile([P, DMODEL], F32)
    eps_sb = keep.tile([P, 1], F32)
    nc.vector.memset(eps_sb, 1e-5)
    resid3 = dram3(resid.ap())
    out3 = dram3(out.ap())

    NST = SQ // P
    mv_all = keep.tile([P, NST, 2], F32)     # per-block (mean, var)
    rstd_all = keep.tile([P, NST, 2], F32)   # (scratch ln, rstd)
    xts = {}

    def stE_pre(st):
        """Output projection + residual + bn stats for query block st."""
        rin = rin_pre[:, st, :]
        x_t = dpool.tile([P, DMODEL], F32, tag="x", bufs=5)
        xts[st] = x_t
        for ec in range(NEC):
            ps = pproj.tile([P, 512], F32, tag="pp")
            for dj in range(DTL // 2):
                nc.tensor.matmul(
                    ps,
                    lhsT=aoT[:, 2 * dj:2 * dj + 2, st * P:(st + 1) * P],
                    rhs=wo_sb[:, 2 * dj:2 * dj + 2,
                              ec * 512:(ec + 1) * 512],
                    start=(dj == 0), stop=(dj == DTL // 2 - 1),
                    perf_mode=DR)
            nc.vector.tensor_add(
                out=x_t[:, ec * 512:(ec + 1) * 512],
                in0=ps, in1=rin[:, ec * 512:(ec + 1) * 512])
        stats = dpool.tile([P, 2, 6], F32, tag="stats", bufs=2)
        nc.vector.bn_stats(out=stats[:, 0, :], in_=x_t[:, 0:512])
        nc.vector.bn_stats(out=stats[:, 1, :], in_=x_t[:, 512:1024])
        nc.vector.bn_aggr(out=mv_all[:, st, :], in_=stats)

    def stE_rstd(lo, hi):
        """rstd = exp(-0.5*ln(var+eps)) for query blocks [lo, hi).
        Ln and Exp live in one activation table set with the softmax
        Exp, so this never forces a table reload mid-stream."""
        nc.scalar.activation(out=rstd_all[:, lo:hi, 0],
                             in_=mv_all[:, lo:hi, 1],
                             func=AF.Ln,
                             bias=eps_sb[:, 0:1], scale=1.0)
        nc.scalar.activation(out=rstd_all[:, lo:hi, 1],
                             in_=rstd_all[:, lo:hi, 0],
                             func=AF.Exp,
                             bias=0.0, scale=-0.5)

    def stE_post(st, on_act=False, dma_eng=None):
        """LayerNorm application + store for query block st.  The
        normalize runs on DVE mid-stream (ScalarE is exp-bound there)
        but on ScalarE for the tail blocks, where it idles and DVE is
        the pole."""
        x_t = xts.pop(st)
        y_t = dpool.tile([P, DMODEL], F32, tag="y", bufs=2)
        if on_act:
            nb = dpool.tile([P, 1], F32, tag="nb", bufs=2)
            nc.vector.tensor_tensor(nb, mv_all[:, st, 0:1],
                                    rstd_all[:, st, 1:2],
                                    mybir.AluOpType.mult)
            nc.vector.tensor_scalar_mul(nb, nb, -1.0)
            nc.scalar.activation(
                out=y_t, in_=x_t,
                func=AF.Identity,
                scale=rstd_all[:, st, 1:2], bias=nb[:, 0:1])
        else:
            nc.vector.tensor_scalar(
                out=y_t, in0=x_t, scalar1=mv_all[:, st, 0:1],
                scalar2=rstd_all[:, st, 1:2],
                op0=mybir.AluOpType.subtract, op1=mybir.AluOpType.mult)
        if not ln_trivial:
            nc.gpsimd.tensor_mul(out=y_t, in0=y_t, in1=gamma_sb)
            nc.gpsimd.tensor_add(out=y_t, in0=y_t, in1=beta_sb)
        (dma_eng or nc.sync).dma_start(out=out3[:, st, :], in_=y_t)

    # ======== prologue: just the two chunks the first exp needs ========
    kth(0, 0, PKC0)()
    qth(0, 0)()
    nc.sync.dma_start(out=wo_sb, in_=dram3(woT.ap()))
    if not ln_trivial:
        nc.gpsimd.dma_start(out=gamma_sb, in_=bass.AP(
            tensor=gamma.ap().tensor, offset=0, ap=[[0, P], [1, DMODEL]]))
        nc.gpsimd.dma_start(out=beta_sb, in_=bass.AP(
            tensor=beta.ap().tensor, offset=0, ap=[[0, P], [1, DMODEL]]))
    rin_pre = keep.tile([P, SQ // P, DMODEL], BF16)
    for st in range(SQ // P):
        nc.sync.dma_start(out=rin_pre[:, st, :], in_=resid3[:, st, :])

    if phases < 4:
        for j in range(NPAIR):
            if PKC0 < K0:
                kth(0, PKC0, K0)() if j == 0 else None
            for c in range(NKC):
                if (j, c) != (0, 0):
                    kth(j, *KCH[c])()
            if j != 0:
                kth(j, 0, PKC0)()
            for c in range(NQCH):
                if (j, c) != (0, 0):
                    qth(j, c)()
            for g in range(NVG):
                vth(j, g)()
        nc.sync.dma_start(out=out.ap()[0:P, 0:LPAD // 2],
                          in_=kts[NPAIR - 1].bitcast(F32))
        nc.sync.dma_start(out=out.ap()[P:2 * P, 0:SQ // 2],
                          in_=qts[NPAIR - 1].bitcast(F32))
        return

    # ---- fill schedule (qc0): everything except the prologue's two
    # thunks, deadline-ordered into the kt slots of each pair iteration.
    # Deadlines (slot = kt index within iteration j):
    #   vth(j,0) before attnv(0) at kt=3; kth(j,c>=2) before scores(8);
    #   vth(j,1)/(j,2) before later attnv; kth(j+1,0)/qth(j+1,0)/
    #   kth(j+1,1) before iteration j+1 needs them; qth(*,1) before qc1.
    fills_qc0 = []
    for j in range(NPAIR):
        kches = []
        if j == 0 and PKC0 < K0:
            kches.append((PKC0, K0))
        kches += [KCH[c] for c in range(1, NKC)]
        vgs = list(range(NVG))
        fl = []
        while kches or vgs:
            if kches:
                fl.append(kth(j, *kches.pop(0)))
            if vgs:
                fl.append(vth(j, vgs.pop(0)))
        if j + 1 < NPAIR:
            fl += [kth(j + 1, 0, K0), qth(j + 1, 0)]
        if j > 0:
            fl += [qth(j - 1, c) for c in range(1, NQCH)]
        if j == NPAIR - 1:
            fl += [qth(j, c) for c in range(1, NQCH)]
        fills_qc0.append(fl)

    # ======== main: qc outer, head-pair inner ========
    # qc0: projections as TensorE filler.  qc1: phase E for qc0's rows as
    # filler.  Epilogues deferred one iteration so ScalarE never waits at
    # iteration boundaries; each iteration's scores(0) is pre-issued at
    # the tail of the previous iteration for the same reason.
    def make_iter(qc, j, last=False):
        """One (query-chunk, head-pair) attention iteration.

        `last`: the final iteration's odd head runs a V-only matmul into
        PSUM partitions 64-127 (base 64) plus a rowsum matmul from the
        ones column (base 32), so its epilogue writes aoT[64:128]
        directly instead of shifting partitions with a DMA — that DMA
        latency would sit on the serial tail."""
        qsl = slice(qc * 512, (qc + 1) * 512)
        avA = pattn.tile([HD + 1, 512], F32, tag="avA")
        avB_t = pattn.tile([P, 512], F32, tag="avB")
        es = {}

        def scores(kt):
            ps = pscore.tile([P, 1024], F32, tag="ps_s")
            nc.tensor.matmul(
                ps[:, 0:512],
                lhsT=kts[j][0:64, kt * P:(kt + 1) * P],
                rhs=qts[j][0:64, qsl], start=True, stop=True)
            nc.tensor.matmul(
                ps[:, 512:1024],
                lhsT=kts[j][64:128, kt * P:(kt + 1) * P],
                rhs=qts[j][64:128, qsl], start=True, stop=True)
            ktp, par = divmod(kt, 2)
            if par == 0:
                e2 = dpool.tile([P, 2, 2, 512], F8, tag="e2", bufs=4)
                es[ktp] = e2
            else:
                e2 = es[ktp]
            nc.scalar.activation(
                out=e2[:, par, :, :],
                in_=ps.rearrange("p (a b) -> p a b", b=512),
                func=AF.Exp,
                bias=maskb_sb[:, kt:kt + 1], scale=0.125)

        hA = 2 * j
        hB = 2 * j + 1

        def attnv(ktp):
            st = (ktp == 0)
            sp = (ktp == NKTP - 1) and not ODD
            if ktp < NKTP:
                e2 = es[ktp]
                vsl = slice(2 * ktp, 2 * ktp + 2)
                nc.tensor.matmul(
                    avA, lhsT=vaug[:, vsl, hA, :],
                    rhs=e2[:, :, 0, :],
                    start=st, stop=sp, perf_mode=DR)
                if not last:
                    nc.tensor.matmul(
                        avB_t[0:HD + 1, :], lhsT=vaug[:, vsl, hB, :],
                        rhs=e2[:, :, 1, :],
                        start=st, stop=sp, perf_mode=DR)
                else:
                    # DoubleRow can't target base partition 64 (column
                    # tiling and DR are mutually exclusive), so the
                    # direct-placement passes run one kt at a time
                    for par in range(2):
                        kt = 2 * ktp + par
                        nc.tensor.matmul(
                            avB_t[64:128, :],
                            lhsT=vaug[:, kt, hB, 0:HD],
                            rhs=e2[:, par, 1, :],
                            start=(st and par == 0),
                            stop=(sp and par == 1))
                        nc.tensor.matmul(
                            avB_t[0:1, :],
                            lhsT=vaug[:, kt, hB, HD:HD + 1],
                            rhs=e2[:, par, 1, :],
                            start=(st and par == 0),
                            stop=(sp and par == 1))
            else:          # leftover single-kt pass (odd KT)
                e2 = es[ktp]
                nc.tensor.matmul(
                    avA, lhsT=vaug[:, KT - 1, hA, :],
                    rhs=e2[:, 0, 0, :],
                    start=False, stop=True)
                if not last:
                    nc.tensor.matmul(
                        avB_t[0:HD + 1, :], lhsT=vaug[:, KT - 1, hB, :],
                        rhs=e2[:, 0, 1, :],
                        start=False, stop=True)
                else:
                    nc.tensor.matmul(
                        avB_t[64:128, :],
                        lhsT=vaug[:, KT - 1, hB, 0:HD],
                        rhs=e2[:, 0, 1, :],
                        start=False, stop=True)
                    nc.tensor.matmul(
                        avB_t[0:1, :],
                        lhsT=vaug[:, KT - 1, hB, HD:HD + 1],
                        rhs=e2[:, 0, 1, :],
                        start=False, stop=True)

        def epilogue():
            # deferred pipeline tail
            if NKTP > 0:
                attnv(NKTP - 1)
            if ODD:
                attnv(NKTP)
            # normalize in place: aoT = av / rowsum, reading the PSUM
            # accumulators directly.  1/rowsum is broadcast across
            # partitions with a K=1 matmul; DVE reads at most one PSUM
            # operand, so the broadcast drains to SBUF first.
            # DVE lanes are hardwired to partitions: reciprocal outputs
            # stay on the rowsum's own partition (64 regular, 32 last)
            rrow = 0 if last else 64
            rsB = avB_t[rrow:rrow + 1, :]
            rc = dpool.tile([P, 1024], F32R, tag="recip", bufs=2)
            with nc.allow_low_precision(
                    reason="f32r recip feeds f32r matmul"):
                nc.vector.reciprocal(
                    out=rc[64:65, 0:512], in_=avA[64:65, :])
                nc.vector.reciprocal(
                    out=rc[rrow:rrow + 1, 512:1024], in_=rsB)
            rbA = pproj.tile([P, 512], F32, tag="pp")
            rbB = pproj.tile([P, 512], F32, tag="pp")
            bsl = slice(64, 128) if last else slice(0, 64)
            nc.tensor.matmul(rbA[0:64, :], lhsT=ones_sb[64:65, 0:64],
                             rhs=rc[64:65, 0:512],
                             start=True, stop=True)
            # `last` broadcasts to all 128 partitions (tile_position
            # (0,0)); a direct base-64 K=1 matmul fails the ISA check
            nc.tensor.matmul(rbB[0:P, :] if last else rbB[0:64, :],
                             lhsT=(ones_sb[0:1, 0:P] if last
                                   else ones_sb[64:65, 0:64]),
                             rhs=rc[rrow:rrow + 1, 512:1024],
                             start=True, stop=True)
            nc.vector.tensor_copy(out=rc[0:64, 0:512], in_=rbA[0:64, :])
            nc.vector.tensor_copy(out=rc[bsl, 512:1024], in_=rbB[bsl, :])
            nc.vector.tensor_tensor(
                aoT[0:64, j, qsl], avA[0:64, :],
                rc[0:64, 0:512].bitcast(F32),
                mybir.AluOpType.mult)
            if last:
                nc.vector.tensor_tensor(
                    aoT[64:128, j, qsl], avB_t[64:128, :],
                    rc[64:128, 512:1024].bitcast(F32),
                    mybir.AluOpType.mult)
            else:
                av_f8 = dpool.tile([64, 512], F8, tag="av_f8", bufs=2)
                nc.vector.tensor_tensor(
                    av_f8, avB_t[0:64, :],
                    rc[0:64, 512:1024].bitcast(F32),
                    mybir.AluOpType.mult)
                nc.sync.dma_start(out=aoT[64:128, j, qsl], in_=av_f8)

        return scores, attnv, epilogue

    iters = [(qc, j) for qc in range(NQC) for j in range(NPAIR)]
    pend = [None]
    cur = make_iter(*iters[0], last=(len(iters) == 1))
    cur[0](0)                      # scores(0) of the first iteration
    stqueue = []
    for idx, (qc, j) in enumerate(iters):
        scores, attnv, epilogue = cur
        if qc == 1 and j == 0:
            # phase E for qc0's blocks, one piece per head-pair slot
            stqueue = [lambda s=s: stE_pre(s) for s in range(NST // 2)]
            stqueue.append(lambda: (stE_rstd(0, NST // 2), stE_post(0)))
            stqueue += [lambda s=s: stE_post(s) for s in range(1, NST // 2)]
        if KT <= 2 and pend[0] is not None:
            pend[0]()
            pend[0] = None
        fill = list(fills_qc0[j]) if qc == 0 else []
        stfill = [stqueue.pop(0)] if (qc == 1 and stqueue) else []
        for kt in range(1, KT):
            scores(kt)
            if kt == min(3, KT - 1) and pend[0] is not None:
                # prev iteration's attnv tail + epilogue: issued behind
                # three scores so its PE-side wait on the DVE reciprocal
                # never starves ScalarE at the boundary
                pend[0]()
                pend[0] = None
            if kt % 2 == 1 and kt >= 3:
                attnv(kt // 2 - 1)
            if fill:
                fill.pop(0)()
            elif stfill and kt >= 4:
                stfill.pop(0)()
        # pre-issue the next iteration's first scores+exp so ScalarE
        # never waits at the boundary
        if idx + 1 < len(iters):
            nxt = make_iter(*iters[idx + 1], last=(idx + 2 == len(iters)))
            nxt[0](0)
        else:
            nxt = None
        while fill:
            fill.pop(0)()
        if nxt is None:
            epilogue()         # last iteration: run now, don't defer
        else:
            pend[0] = epilogue
        cur = nxt

    if pend[0] is not None:
        pend[0]()
        pend[0] = None

    if phases < 5:
        return

    # phase E tail: qc1's query blocks (qc0's ran as qc1 attention
    # fills).  ScalarE is idle after the last exp, so the normalize runs
    # there (on_act) while DVE works the adds/stats chain; out DMAs
    # alternate queues so the stores overlap.
    h = NST // 2
    stE_pre(h)
    stE_pre(h + 1)
    stE_rstd(h, h + 1)
    stE_post(h, on_act=True)
    stE_pre(h + 2)
    stE_rstd(h + 1, h + 2)
    stE_post(h + 1, on_act=True, dma_eng=nc.gpsimd)
    stE_pre(h + 3)
    stE_rstd(h + 2, h + 3)
    stE_post(h + 2, on_act=True)
    stE_rstd(h + 3, h + 4)
    stE_post(h + 3, on_act=True, dma_eng=nc.gpsimd)


_cache = {}


def _get_nc(LPAD, ln_trivial):
    key = (LPAD, ln_trivial)
    if key not in _cache:
        _cache[key] = _build(LPAD, ln_trivial)
    return _cache[key]


def make_in_maps(query, key, value, mask, Wq, Wk, Wv, Wo, ln_gamma, ln_beta):
    """Host-side sharding: returns (in_maps, LPAD, ln_trivial)."""
    f = lambda a: np.ascontiguousarray(np.asarray(a, np.float32))
    f8 = lambda a: np.ascontiguousarray(
        np.asarray(np.asarray(a, np.float32), NPF8))
    query, key, value = f(query), f(key), f(value)
    mask = np.asarray(mask)
    wqT = f8(np.asarray(Wq, np.float32).T)
    wkT = f8(np.asarray(Wk, np.float32).T)
    wvT = f8(np.asarray(Wv, np.float32).T)
    woT = f8(np.asarray(Wo, np.float32).T)
    gamma, beta = f(ln_gamma), f(ln_beta)
    ln_trivial = bool(np.all(gamma == 1.0) and np.all(beta == 0.0))

    idxs = []
    for b in range(B):
        ix = np.nonzero(mask[b] != 0)[0]
        if len(ix) == 0:
            # all-masked row: the -1e8 bias is common to every key, so the
            # reference softmax reduces to plain softmax over all keys.
            ix = np.arange(S)
        idxs.append(ix)
    Lmax = max(len(ix) for ix in idxs)
    LPAD = max(2 * P, ((Lmax + P - 1) // P) * P)
    KT = LPAD // P

    in_maps = []
    for c in range(NCORES):
        b, g = divmod(c, 2)
        ix = idxs[b]
        L = len(ix)
        kc = np.zeros((LPAD, DMODEL), np.float32)
        kc[:L] = key[b][ix]
        vc = np.zeros((LPAD, DMODEL), np.float32)
        vc[:L] = value[b][ix]
        # -4.5 recenters exp so fp8 e2 stays below e4m3 max 240 (scores
        # reach ~9.3, exp overflows past 4.5+ln(240)=9.98); the shift is
        # uniform over keys so it cancels in normalization
        mb = np.full((LPAD,), -1e30, np.float32)
        mb[:L] = -4.5
        qrows = query[b, g * SQ:(g + 1) * SQ]
        in_maps.append({
            "qT": f8(qrows.T),
            "kT": f8(kc.T),
            "vT": f8(vc.T),
            "resid": np.ascontiguousarray(
                qrows.astype(ml_dtypes.bfloat16)),
            "wqT": wqT, "wkT": wkT, "wvT": wvT, "woT": woT,
            "maskb": np.ascontiguousarray(mb.reshape(KT, P).T),
            "gamma": gamma, "beta": beta,
        })
    return in_maps, LPAD, ln_trivial


def gather_out(results):
    out = np.empty((B, S, DMODEL), np.float32)
    for c in range(NCORES):
        b, g = divmod(c, 2)
        out[b, g * SQ:(g + 1) * SQ] = results[c]["out"]
    return out


def kernel(query, key, value, mask, Wq, Wk, Wv, Wo, ln_gamma, ln_beta):
    in_maps, LPAD, ln_trivial = make_in_maps(
        query, key, value, mask, Wq, Wk, Wv, Wo, ln_gamma, ln_beta)
    nc = _get_nc(LPAD, ln_trivial)
    res = run_bass_kernel_spmd(nc, in_maps, list(range(NCORES)))
    return gather_out(res.results)


# revision 21
# speedup vs baseline: 1.5137x; 1.5137x over previous
"""Trainium2 Bass kernel for a post-LN multi-head-attention block.

Reference computation (B=4, S=2048, D=1024, 16 heads x 64):
    q,k,v = x @ W{q,k,v}.T ; attn = softmax(q k^T/8 + mask) ; o = attn v
    out = LayerNorm(query + (o @ Wo.T)) * gamma + beta

Sharding: 8 cores = 4 batches x 2 query-halves (1024 query rows per core).
Each core computes all 16 heads for its query rows against the full
(mask-compacted) key set of its batch.  No collectives.

Key implementation choices:
  - keys with mask==0 contribute exactly 0 attention weight, so the host
    compacts key/value to the unmasked subset, padded to a multiple of 128
    (padding biased -1e30 so exp -> 0).
  - Q/K/V/O projections and attnV run in fp8e4m3 with DoubleRow perf
    mode (2 contraction rows per PE pass = 2x f32r/bf16 matmul
    throughput).  The attention output is only ~5% of the residual
    magnitude entering LayerNorm, so fp8 noise dilutes ~20x in the final
    output (measured end-to-end rel err ~9e-3 vs the 2e-2 budget).
  - scores keep fp16 operands; exp bias includes -4.5 so fp8
    e2 = exp(qk/8 - 4.5) stays below e4m3 max 240.
  - scores are computed transposed, scoresT[k, q], so softmax's
    k-reduction becomes a matmul reduction: V is augmented with a
    ones-column and attnV produces [out^T ; rowsum] in one PSUM group.
    Even heads use a [V | 1] augmentation (accumulator rows 0-64), odd
    heads a [1 | V] augmentation placed at PSUM partitions 63-127, so
    the normalized output lands directly in aoT partitions 64-127
    without a partition-shift DMA.
  - ScalarE (softmax exp) is the bottleneck engine (~137us of exp at
    1 elem/cycle/lane); everything else is scheduled around keeping it
    saturated: the prologue only computes the two projection chunks the
    first exp needs (their DMAs spread over four DGE queues), all other
    projections are deadline-ordered fills inside the kt loops, each
    iteration's first scores matmul is pre-issued at the tail of the
    previous iteration, and LayerNorm rstd uses Ln+Exp (one activation
    table set with the softmax Exp -> no table reloads).
  - loop nest is qc-outer / head-pair-inner: projections fill the
    TensorE during qc0's attention, and the output projection +
    LayerNorm for qc0 overlaps qc1's attention; the qc1 epilogue
    applies LayerNorm on ScalarE (idle after the last exp).
  - gamma/beta are ones/zeros in this problem; make_in_maps verifies at
    runtime and compiles the gamma/beta application out (falling back to
    a gpsimd apply if they were ever nontrivial).
"""

import numpy as np
import ml_dtypes

import concourse.bacc as bacc
import concourse.tile as tile
import concourse.bass as bass
from concourse import mybir
from concourse.bass_utils import run_bass_kernel_spmd

DMODEL = 1024
NHEAD = 16
HD = 64
B = 4
S = 2048
NCORES = 8
SQ = 1024          # query rows per core
P = 128
F32 = mybir.dt.float32
F32R = mybir.dt.float32r
F16 = mybir.dt.float16
BF16 = mybir.dt.bfloat16
F8 = mybir.dt.float8e4
DR = mybir.MatmulPerfMode.DoubleRow
NPF8 = ml_dtypes.float8_e4m3
ET = DMODEL // P   # 8 e-tiles (feature tiles)
DTL = DMODEL // P  # 8 d-tiles (contraction tiles)
NQC = SQ // 512    # 2 query chunks of 512
NEC = DMODEL // 512  # 2 feature chunks of 512
NPAIR = NHEAD // 2   # 8 head pairs; pair j = heads (2j, 2j+1) in e-tile j
AF = mybir.ActivationFunctionType


def _balanced_chunks(total, maxw=512):
    """Split `total` (a multiple of 128) into maximal chunks <= maxw,
    widest first: 512B-contiguous DMA runs dodge the sub-512B 2x
    latency penalty."""
    out, lo = [], 0
    while lo < total:
        w = min(maxw, total - lo)
        out.append((lo, lo + w))
        lo += w
    return out


def _build(LPAD, ln_trivial=True, do_compile=True, reps=1, phases=5):
    KT = LPAD // P
    KCH = _balanced_chunks(LPAD)
    QCH = _balanced_chunks(SQ)
    nc = bacc.Bacc("TRN2", target_bir_lowering=False, debug=False,
                   num_devices=NCORES)

    qT = nc.declare_dram_parameter("qT", [DMODEL, SQ], F8, isOutput=False)
    kT = nc.declare_dram_parameter("kT", [DMODEL, LPAD], F8, isOutput=False)
    vT = nc.declare_dram_parameter("vT", [DMODEL, LPAD], F8, isOutput=False)
    resid = nc.declare_dram_parameter("resid", [SQ, DMODEL], BF16,
                                      isOutput=False)
    wqT = nc.declare_dram_parameter("wqT", [DMODEL, DMODEL], F8, isOutput=False)
    wkT = nc.declare_dram_parameter("wkT", [DMODEL, DMODEL], F8, isOutput=False)
    wvT = nc.declare_dram_parameter("wvT", [DMODEL, DMODEL], F8, isOutput=False)
    woT = nc.declare_dram_parameter("woT", [DMODEL, DMODEL], F8, isOutput=False)
    maskb = nc.declare_dram_parameter("maskb", [P, KT], F32, isOutput=False)
    gamma = nc.declare_dram_parameter("gamma", [DMODEL], F32, isOutput=False)
    beta = nc.declare_dram_parameter("beta", [DMODEL], F32, isOutput=False)
    out = nc.declare_dram_parameter("out", [SQ, DMODEL], F32, isOutput=True)

    def dram3(ap):
        # (o*P, width) DRAM tensor viewed as [p, o, width]
        return ap.rearrange("(o p) w -> p o w", p=P)

    with tile.TileContext(nc) as tc:
        with (
            tc.tile_pool(name="keep", bufs=1) as keep,      # long-lived SBUF
            tc.tile_pool(name="dpool", bufs=2) as dpool,
            tc.tile_pool(name="pproj", bufs=2, space="PSUM") as pproj,
            tc.tile_pool(name="pscore", bufs=2, space="PSUM") as pscore,
            tc.tile_pool(name="pattn", bufs=1, space="PSUM") as pattn,
        ):
            for _rep in range(reps):
                self_body(nc, tc, keep, dpool, pproj, pscore, pattn, phases,
                          ln_trivial, LPAD, KT, KCH, QCH,
                          qT, kT, vT, resid, wqT, wkT, wvT, woT,
                          maskb, gamma, beta, out, dram3)
    if do_compile:
        nc.compile()
    return nc


def self_body(nc, tc, keep, dpool, pproj, pscore, pattn, phases, ln_trivial,
              LPAD, KT, KCH, QCH,
              qT, kT, vT, resid, wqT, wkT, wvT, woT, maskb, gamma, beta,
              out, dram3):
    NKTP = KT // 2            # full DoubleRow kt-pairs in attnV
    ODD = KT % 2
    NKC = len(KCH)
    NQCH = len(QCH)

    # ---- long-lived tensors ----
    vaug = keep.tile([P, KT, NHEAD, HD + 1], F8)     # [k | head | V,1]
    aoT = keep.tile([P, ET, SQ], F8)                 # attn out^T (d' on part)
    maskb_sb = keep.tile([P, KT], F32)
    nc.sync.dma_start(out=maskb_sb, in_=maskb.ap())
    ones_sb = keep.tile([65, 128], F32R)
    nc.vector.memset(ones_sb[64:65, :].bitcast(F32), 1.0)
    nc.vector.memset(ones_sb[0:1, :].bitcast(F32), 1.0)
    nc.vector.memset(vaug[:, :, :, HD:HD + 1], 1.0)
    kts = [keep.tile([P, LPAD], F16, name=f"kts{j}")
           for j in range(NPAIR)]
    qts = [keep.tile([P, SQ], F16, name=f"qts{j}")
           for j in range(NPAIR)]

    vT3 = dram3(vT.ap())
    wvT3 = dram3(wvT.ap())
    kT3 = dram3(kT.ap())
    qT3 = dram3(qT.ap())
    wkT3 = dram3(wkT.ap())
    wqT3 = dram3(wqT.ap())
    wk_sb = keep.tile([P, DTL, DMODEL], F8)
    wq_sb = keep.tile([P, DTL, DMODEL], F8)
    wv_sb = keep.tile([P, DTL, DMODEL], F8)
    kin = keep.tile([P, DTL, LPAD], F8)
    qin = keep.tile([P, DTL, SQ], F8)
    vin = keep.tile([P, DTL, LPAD], F8)
    K0 = KCH[0][1]
    Q0 = QCH[0][1]
    PKC0 = K0
    W0 = 512               # weight slice covering pairs 0-3
    # DMA bandwidth is one shared resource, so queue parallelism cannot
    # help the ramp -- ordering can.  Everything goes on the SP queue in
    # strict first-use order: the four transfers the first exp needs
    # (pair-0 weight columns, first key/query chunks) lead the stream.
    # DMA bandwidth is one shared resource and sub-512B-contiguous
    # slices pay a 2x latency penalty, so the stream is ordered by first
    # use with slice widths chosen around that penalty: 128-col weight
    # slices unblock pair 0/1 early (narrow but short), key/query chunks
    # stay >=512B, and the wide remainders stream at full rate.
    W1 = min(2 * P, DMODEL)
    nc.sync.dma_start(out=wk_sb[:, :, 0:P], in_=wkT3[:, :, 0:P])
    nc.sync.dma_start(out=kin[:, :, 0:K0], in_=kT3[:, :, 0:K0])
    nc.sync.dma_start(out=wq_sb[:, :, 0:P], in_=wqT3[:, :, 0:P])
    nc.sync.dma_start(out=qin[:, :, 0:Q0], in_=qT3[:, :, 0:Q0])
    if K0 < LPAD:
        nc.sync.dma_start(out=kin[:, :, K0:LPAD], in_=kT3[:, :, K0:LPAD])
    nc.sync.dma_start(out=wk_sb[:, :, P:W1], in_=wkT3[:, :, P:W1])
    nc.sync.dma_start(out=wq_sb[:, :, P:W1], in_=wqT3[:, :, P:W1])
    nc.sync.dma_start(out=wv_sb[:, :, 0:P], in_=wvT3[:, :, 0:P])
    nc.sync.dma_start(out=vin[:, :, 0:K0], in_=vT3[:, :, 0:K0])
    if K0 < LPAD:
        nc.sync.dma_start(out=vin[:, :, K0:LPAD], in_=vT3[:, :, K0:LPAD])
    nc.sync.dma_start(out=wv_sb[:, :, P:W1], in_=wvT3[:, :, P:W1])
    nc.sync.dma_start(out=wk_sb[:, :, W1:DMODEL], in_=wkT3[:, :, W1:DMODEL])
    nc.sync.dma_start(out=wq_sb[:, :, W1:DMODEL], in_=wqT3[:, :, W1:DMODEL])
    nc.sync.dma_start(out=wv_sb[:, :, W1:DMODEL], in_=wvT3[:, :, W1:DMODEL])
    for (lo, hi) in QCH[1:]:
        nc.sync.dma_start(out=qin[:, :, lo:hi], in_=qT3[:, :, lo:hi])

    def kth(j, lo, hi):
        """K^T projection thunk: kts[j][:, lo:hi] (fp8 DoubleRow, drained
        straight to fp16)."""
        def f(j=j, lo=lo, hi=hi):
            w = hi - lo
            ps = pproj.tile([P, 512], F32, tag="pp")
            for dt in range(DTL // 2):
                nc.tensor.matmul(
                    ps[:, :w],
                    lhsT=wk_sb[:, 2 * dt:2 * dt + 2, j * P:(j + 1) * P],
                    rhs=kin[:, 2 * dt:2 * dt + 2, lo:hi],
                    start=(dt == 0), stop=(dt == DTL // 2 - 1),
                    perf_mode=DR)
            nc.vector.tensor_copy(out=kts[j][:, lo:hi], in_=ps[:, :w])
        return f

    def qth(j, c):
        """Q^T projection thunk: qts[j][:, QCH[c]]."""
        def f(j=j, c=c):
            lo, hi = QCH[c]
            w = hi - lo
            ps = pproj.tile([P, 512], F32, tag="pp")
            for dt in range(DTL // 2):
                nc.tensor.matmul(
                    ps[:, :w],
                    lhsT=wq_sb[:, 2 * dt:2 * dt + 2, j * P:(j + 1) * P],
                    rhs=qin[:, 2 * dt:2 * dt + 2, lo:hi],
                    start=(dt == 0), stop=(dt == DTL // 2 - 1),
                    perf_mode=DR)
            nc.vector.tensor_copy(out=qts[j][:, lo:hi], in_=ps[:, :w])
        return f

    # V-projection per pair: kt-blocks grouped ~3 per thunk so a pair's
    # V work fits one fill slot
    _vstep = min((KT + 2) // 3, 4)
    VGRP = [(a, min(a + _vstep, KT)) for a in range(0, KT, _vstep)]
    NVG = len(VGRP)

    def vth(j, g):
        def f(j=j, g=g):
            g0, g1 = VGRP[g]
            gl = g1 - g0
            ps = pproj.tile([P, 512], F32, tag="pp")
            for i in range(gl):
                for dt in range(DTL // 2):
                    nc.tensor.matmul(
                        ps[:, i * P:(i + 1) * P],
                        lhsT=vin[:, 2 * dt:2 * dt + 2,
                                 (g0 + i) * P:(g0 + i + 1) * P],
                        rhs=wv_sb[:, 2 * dt:2 * dt + 2,
                                  j * P:(j + 1) * P],
                        start=(dt == 0), stop=(dt == DTL // 2 - 1),
                        perf_mode=DR)
            nc.vector.tensor_copy(
                out=vaug[:, g0:g1, 2 * j:2 * j + 2, 0:HD],
                in_=ps[:, 0:gl * P].rearrange(
                    "p (k h x) -> p k h x", h=2, x=HD))
        return f

    # ---- phase E pieces (output proj + residual + layernorm);
    # wo DMA is issued after the prologue so it doesn't delay vin ----
    wo_sb = keep.tile([P, DTL, DMODEL], F8)
    if not ln_trivial:
        gamma_sb = keep.tile([P, DMODEL], F32)
        beta_sb = keep.tile([P, DMODEL], F32)
    eps_sb = keep.tile([P, 1], F32)
    nc.vector.memset(eps_sb, 1e-5)
    resid3 = dram3(resid.ap())
    out3 = dram3(out.ap())

    NST = SQ // P
    mv_all = keep.tile([P, NST, 2], F32)     # per-block (mean, var)
    rstd_all = keep.tile([P, NST, 2], F32)   # (scratch ln, rstd)
    xts = {}

    def stE_pre(st):
        """Output projection + residual + bn stats for query block st."""
        rin = rin_pre[:, st, :]
        x_t = dpool.tile([P, DMODEL], F32, tag="x", bufs=5)
        xts[st] = x_t
        for ec in range(NEC):
            ps = pproj.tile([P, 512], F32, tag="pp")
            for dj in range(DTL // 2):
                nc.tensor.matmul(
                    ps,
                    lhsT=aoT[:, 2 * dj:2 * dj + 2, st * P:(st + 1) * P],
                    rhs=wo_sb[:, 2 * dj:2 * dj + 2,
                              ec * 512:(ec + 1) * 512],
                    start=(dj == 0), stop=(dj == DTL // 2 - 1),
                    perf_mode=DR)
            nc.vector.tensor_add(
                out=x_t[:, ec * 512:(ec + 1) * 512],
                in0=ps, in1=rin[:, ec * 512:(ec + 1) * 512])
        stats = dpool.tile([P, 2, 6], F32, tag="stats", bufs=2)
        nc.vector.bn_stats(out=stats[:, 0, :], in_=x_t[:, 0:512])
        nc.vector.bn_stats(out=stats[:, 1, :], in_=x_t[:, 512:1024])
        nc.vector.bn_aggr(out=mv_all[:, st, :], in_=stats)

    magic_sb = keep.tile([P, NST], mybir.dt.int32)
    nc.vector.memset(magic_sb, 0x5F3759DF)

    def stE_rstd(lo, hi):
        """rstd = 1/sqrt(var+eps) for query blocks [lo, hi) via the
        fast-inverse-square-root bit trick + 2 Newton steps, on DVE.
        Using ScalarE's Sqrt here would evict the softmax Exp activation
        table (~2.7us reload each way); the DVE ops are only a few
        elements per partition."""
        n = hi - lo
        t = rstd_all[:, lo:hi, 0]
        y = rstd_all[:, lo:hi, 1]
        a_t = dpool.tile([P, NST], F32, tag="nwt", bufs=2)
        a = a_t[:, 0:n]
        eng = nc.vector
        eng.tensor_scalar_add(t, mv_all[:, lo:hi, 1], 1e-5)
        eng.tensor_scalar(y.bitcast(mybir.dt.int32), t.bitcast(
            mybir.dt.int32), 1, None, op0=mybir.AluOpType.arith_shift_right)
        eng.tensor_tensor(y.bitcast(mybir.dt.int32), magic_sb[:, 0:n],
                          y.bitcast(mybir.dt.int32),
                          mybir.AluOpType.subtract)
        for _ in range(2):
            eng.tensor_tensor(a, y, y, mybir.AluOpType.mult)
            eng.tensor_tensor(a, a, t, mybir.AluOpType.mult)
            eng.tensor_scalar(a, a, -0.5, 1.5, op0=mybir.AluOpType.mult,
                              op1=mybir.AluOpType.add)
            eng.tensor_tensor(y, y, a, mybir.AluOpType.mult)

    def stE_post(st, on_act=False, dma_eng=None):
        """LayerNorm application + store for query block st.  The
        normalize runs on DVE mid-stream (ScalarE is exp-bound there)
        but on ScalarE for the tail blocks, where it idles and DVE is
        the pole."""
        x_t = xts.pop(st)
        y_t = dpool.tile([P, DMODEL], F32, tag="y", bufs=2)
        if on_act:
            nb = dpool.tile([P, 1], F32, tag="nb", bufs=2)
            nc.vector.tensor_tensor(nb, mv_all[:, st, 0:1],
                                    rstd_all[:, st, 1:2],
                                    mybir.AluOpType.mult)
            nc.vector.tensor_scalar_mul(nb, nb, -1.0)
            nc.scalar.activation(
                out=y_t, in_=x_t,
                func=AF.Identity,
                scale=rstd_all[:, st, 1:2], bias=nb[:, 0:1])
        else:
            nc.vector.tensor_scalar(
                out=y_t, in0=x_t, scalar1=mv_all[:, st, 0:1],
                scalar2=rstd_all[:, st, 1:2],
                op0=mybir.AluOpType.subtract, op1=mybir.AluOpType.mult)
        if not ln_trivial:
            nc.gpsimd.tensor_mul(out=y_t, in0=y_t, in1=gamma_sb)
            nc.gpsimd.tensor_add(out=y_t, in0=y_t, in1=beta_sb)
        (dma_eng or nc.sync).dma_start(out=out3[:, st, :], in_=y_t)

    # ======== prologue: just the two chunks the first exp needs ========
    kth(0, 0, PKC0)()
    qth(0, 0)()
    nc.sync.dma_start(out=wo_sb, in_=dram3(woT.ap()))
    if not ln_trivial:
        nc.gpsimd.dma_start(out=gamma_sb, in_=bass.AP(
            tensor=gamma.ap().tensor, offset=0, ap=[[0, P], [1, DMODEL]]))
        nc.gpsimd.dma_start(out=beta_sb, in_=bass.AP(
            tensor=beta.ap().tensor, offset=0, ap=[[0, P], [1, DMODEL]]))
    rin_pre = keep.tile([P, SQ // P, DMODEL], BF16)
    for st in range(SQ // P):
        nc.sync.dma_start(out=rin_pre[:, st, :], in_=resid3[:, st, :])

    if phases < 4:
        for j in range(NPAIR):
            if PKC0 < K0:
                kth(0, PKC0, K0)() if j == 0 else None
            for c in range(NKC):
                if (j, c) != (0, 0):
                    kth(j, *KCH[c])()
            if j != 0:
                kth(j, 0, PKC0)()
            for c in range(NQCH):
                if (j, c) != (0, 0):
                    qth(j, c)()
            for g in range(NVG):
                vth(j, g)()
        nc.sync.dma_start(out=out.ap()[0:P, 0:LPAD // 2],
                          in_=kts[NPAIR - 1].bitcast(F32))
        nc.sync.dma_start(out=out.ap()[P:2 * P, 0:SQ // 2],
                          in_=qts[NPAIR - 1].bitcast(F32))
        return

    # ---- fill schedule (qc0): everything except the prologue's two
    # thunks, deadline-ordered into the kt slots of each pair iteration.
    # Deadlines (slot = kt index within iteration j):
    #   vth(j,0) before attnv(0) at kt=3; kth(j,c>=2) before scores(8);
    #   vth(j,1)/(j,2) before later attnv; kth(j+1,0)/qth(j+1,0)/
    #   kth(j+1,1) before iteration j+1 needs them; qth(*,1) before qc1.
    fills_qc0 = []
    for j in range(NPAIR):
        kches = []
        if j == 0 and PKC0 < K0:
            kches.append((PKC0, K0))
        kches += [KCH[c] for c in range(1, NKC)]
        vgs = list(range(NVG))
        fl = []
        while kches or vgs:
            if kches:
                fl.append(kth(j, *kches.pop(0)))
            if vgs:
                fl.append(vth(j, vgs.pop(0)))
        if j + 1 < NPAIR:
            fl += [kth(j + 1, 0, K0), qth(j + 1, 0)]
        if j > 0:
            fl += [qth(j - 1, c) for c in range(1, NQCH)]
        if j == NPAIR - 1:
            fl += [qth(j, c) for c in range(1, NQCH)]
        fills_qc0.append(fl)

    # ======== main: qc outer, head-pair inner ========
    # qc0: projections as TensorE filler.  qc1: phase E for qc0's rows as
    # filler.  Epilogues deferred one iteration so ScalarE never waits at
    # iteration boundaries; each iteration's scores(0) is pre-issued at
    # the tail of the previous iteration for the same reason.
    def make_iter(qc, j, last=False):
        """One (query-chunk, head-pair) attention iteration.

        `last`: the final iteration's odd head runs a V-only matmul into
        PSUM partitions 64-127 (base 64) plus a rowsum matmul from the
        ones column (base 32), so its epilogue writes aoT[64:128]
        directly instead of shifting partitions with a DMA — that DMA
        latency would sit on the serial tail."""
        qsl = slice(qc * 512, (qc + 1) * 512)
        avA = pattn.tile([HD + 1, 512], F32, tag="avA")
        avB_t = pattn.tile([P, 512], F32, tag="avB")
        es = {}

        def scores(kt):
            ps = pscore.tile([P, 1024], F32, tag="ps_s")
            nc.tensor.matmul(
                ps[:, 0:512],
                lhsT=kts[j][0:64, kt * P:(kt + 1) * P],
                rhs=qts[j][0:64, qsl], start=True, stop=True)
            nc.tensor.matmul(
                ps[:, 512:1024],
                lhsT=kts[j][64:128, kt * P:(kt + 1) * P],
                rhs=qts[j][64:128, qsl], start=True, stop=True)
            ktp, par = divmod(kt, 2)
            if par == 0:
                e2 = dpool.tile([P, 2, 2, 512], F8, tag="e2", bufs=4)
                es[ktp] = e2
            else:
                e2 = es[ktp]
            nc.scalar.activation(
                out=e2[:, par, :, :],
                in_=ps.rearrange("p (a b) -> p a b", b=512),
                func=AF.Exp,
                bias=maskb_sb[:, kt:kt + 1], scale=0.125)

        hA = 2 * j
        hB = 2 * j + 1

        def attnv(ktp):
            st = (ktp == 0)
            sp = (ktp == NKTP - 1) and not ODD
            if ktp < NKTP:
                e2 = es[ktp]
                vsl = slice(2 * ktp, 2 * ktp + 2)
                nc.tensor.matmul(
                    avA, lhsT=vaug[:, vsl, hA, :],
                    rhs=e2[:, :, 0, :],
                    start=st, stop=sp, perf_mode=DR)
                if not last:
                    nc.tensor.matmul(
                        avB_t[0:HD + 1, :], lhsT=vaug[:, vsl, hB, :],
                        rhs=e2[:, :, 1, :],
                        start=st, stop=sp, perf_mode=DR)
                else:
                    # DoubleRow can't target base partition 64 (column
                    # tiling and DR are mutually exclusive), so the
                    # direct-placement passes run one kt at a time
                    for par in range(2):
                        kt = 2 * ktp + par
                        nc.tensor.matmul(
                            avB_t[64:128, :],
                            lhsT=vaug[:, kt, hB, 0:HD],
                            rhs=e2[:, par, 1, :],
                            start=(st and par == 0),
                            stop=(sp and par == 1))
                        nc.tensor.matmul(
                            avB_t[0:1, :],
                            lhsT=vaug[:, kt, hB, HD:HD + 1],
                            rhs=e2[:, par, 1, :],
                            start=(st and par == 0),
                            stop=(sp and par == 1))
            else:          # leftover single-kt pass (odd KT)
                e2 = es[ktp]
                nc.tensor.matmul(
                    avA, lhsT=vaug[:, KT - 1, hA, :],
                    rhs=e2[:, 0, 0, :],
                    start=False, stop=True)
                if not last:
                    nc.tensor.matmul(
                        avB_t[0:HD + 1, :], lhsT=vaug[:, KT - 1, hB, :],
                        rhs=e2[:, 0, 1, :],
                        start=False, stop=True)
                else:
                    nc.tensor.matmul(
                        avB_t[64:128, :],
                        lhsT=vaug[:, KT - 1, hB, 0:HD],
                        rhs=e2[:, 0, 1, :],
                        start=False, stop=True)
                    nc.tensor.matmul(
                        avB_t[0:1, :],
                        lhsT=vaug[:, KT - 1, hB, HD:HD + 1],
                        rhs=e2[:, 0, 1, :],
                        start=False, stop=True)

        def epilogue():
            # deferred pipeline tail
            if NKTP > 0:
                attnv(NKTP - 1)
            if ODD:
                attnv(NKTP)
            # normalize in place: aoT = av / rowsum, reading the PSUM
            # accumulators directly.  1/rowsum is broadcast across
            # partitions with a K=1 matmul; DVE reads at most one PSUM
            # operand, so the broadcast drains to SBUF first.
            # DVE lanes are hardwired to partitions: reciprocal outputs
            # stay on the rowsum's own partition (64 regular, 32 last)
            rrow = 0 if last else 64
            rsB = avB_t[rrow:rrow + 1, :]
            rc = dpool.tile([P, 1024], F32R, tag="recip", bufs=2)
            with nc.allow_low_precision(
                    reason="f32r recip feeds f32r matmul"):
                nc.vector.reciprocal(
                    out=rc[64:65, 0:512], in_=avA[64:65, :])
                nc.vector.reciprocal(
                    out=rc[rrow:rrow + 1, 512:1024], in_=rsB)
            rbA = pproj.tile([P, 512], F32, tag="pp")
            rbB = pproj.tile([P, 512], F32, tag="pp")
            bsl = slice(64, 128) if last else slice(0, 64)
            nc.tensor.matmul(rbA[0:64, :], lhsT=ones_sb[64:65, 0:64],
                             rhs=rc[64:65, 0:512],
                             start=True, stop=True)
            # `last` broadcasts to all 128 partitions (tile_position
            # (0,0)); a direct base-64 K=1 matmul fails the ISA check
            nc.tensor.matmul(rbB[0:P, :] if last else rbB[0:64, :],
                             lhsT=(ones_sb[0:1, 0:P] if last
                                   else ones_sb[64:65, 0:64]),
                             rhs=rc[rrow:rrow + 1, 512:1024],
                             start=True, stop=True)
            nc.vector.tensor_copy(out=rc[0:64, 0:512], in_=rbA[0:64, :])
            nc.vector.tensor_copy(out=rc[bsl, 512:1024], in_=rbB[bsl, :])
            nc.vector.tensor_tensor(
                aoT[0:64, j, qsl], avA[0:64, :],
                rc[0:64, 0:512].bitcast(F32),
                mybir.AluOpType.mult)
            if last:
                nc.vector.tensor_tensor(
                    aoT[64:128, j, qsl], avB_t[64:128, :],
                    rc[64:128, 512:1024].bitcast(F32),
                    mybir.AluOpType.mult)
            else:
                av_f8 = dpool.tile([64, 512], F8, tag="av_f8", bufs=2)
                nc.vector.tensor_tensor(
                    av_f8, avB_t[0:64, :],
                    rc[0:64, 512:1024].bitcast(F32),
                    mybir.AluOpType.mult)
                nc.sync.dma_start(out=aoT[64:128, j, qsl], in_=av_f8)

        return scores, attnv, epilogue

    iters = [(qc, j) for qc in range(NQC) for j in range(NPAIR)]
    pend = [None]
    cur = make_iter(*iters[0], last=(len(iters) == 1))
    cur[0](0)                      # scores(0) of the first iteration
    stqueue = []
    for idx, (qc, j) in enumerate(iters):
        scores, attnv, epilogue = cur
        if qc == 1 and j == 0:
            # phase E for qc0's blocks, one piece per head-pair slot
            stqueue = [lambda s=s: stE_pre(s) for s in range(NST // 2)]
            stqueue.append(lambda: (stE_rstd(0, NST // 2), stE_post(0)))
            stqueue += [lambda s=s: stE_post(s) for s in range(1, NST // 2)]
        if KT <= 2 and pend[0] is not None:
            pend[0]()
            pend[0] = None
        fill = list(fills_qc0[j]) if qc == 0 else []
        stfill = [stqueue.pop(0)] if (qc == 1 and stqueue) else []
        for kt in range(1, KT):
            scores(kt)
            if kt == min(3, KT - 1) and pend[0] is not None:
                # prev iteration's attnv tail + epilogue: issued behind
                # three scores so its PE-side wait on the DVE reciprocal
                # never starves ScalarE at the boundary
                pend[0]()
                pend[0] = None
            if kt % 2 == 1 and kt >= 3:
                attnv(kt // 2 - 1)
            if fill:
                fill.pop(0)()
            elif stfill and kt >= 4:
                stfill.pop(0)()
        # pre-issue the next iteration's first scores+exp so ScalarE
        # never waits at the boundary
        if idx + 1 < len(iters):
            nxt = make_iter(*iters[idx + 1], last=(idx + 2 == len(iters)))
            nxt[0](0)
        else:
            nxt = None
        while fill:
            fill.pop(0)()
        if nxt is None:
            epilogue()         # last iteration: run now, don't defer
        else:
            pend[0] = epilogue
        cur = nxt

    if pend[0] is not None:
        pend[0]()
        pend[0] = None

    if phases < 5:
        return

    # phase E tail: qc1's query blocks (qc0's ran as qc1 attention
    # fills).  ScalarE is idle after the last exp, so the normalize runs
    # there (on_act) while DVE works the adds/stats chain; out DMAs
    # alternate queues so the stores overlap.
    h = NST // 2
    stE_pre(h)
    stE_pre(h + 1)
    stE_rstd(h, h + 1)
    stE_post(h, on_act=True)
    stE_pre(h + 2)
    stE_rstd(h + 1, h + 2)
    stE_post(h + 1, on_act=True)
    stE_pre(h + 3)
    stE_rstd(h + 2, h + 3)
    stE_post(h + 2, on_act=True)
    stE_rstd(h + 3, h + 4)
    stE_post(h + 3, on_act=True)


_cache = {}


def _get_nc(LPAD, ln_trivial):
    key = (LPAD, ln_trivial)
    if key not in _cache:
        _cache[key] = _build(LPAD, ln_trivial)
    return _cache[key]


def make_in_maps(query, key, value, mask, Wq, Wk, Wv, Wo, ln_gamma, ln_beta):
    """Host-side sharding: returns (in_maps, LPAD, ln_trivial)."""
    f = lambda a: np.ascontiguousarray(np.asarray(a, np.float32))
    f8 = lambda a: np.ascontiguousarray(
        np.asarray(np.asarray(a, np.float32), NPF8))
    query, key, value = f(query), f(key), f(value)
    mask = np.asarray(mask)
    wqT = f8(np.asarray(Wq, np.float32).T)
    wkT = f8(np.asarray(Wk, np.float32).T)
    wvT = f8(np.asarray(Wv, np.float32).T)
    woT = f8(np.asarray(Wo, np.float32).T)
    gamma, beta = f(ln_gamma), f(ln_beta)
    ln_trivial = bool(np.all(gamma == 1.0) and np.all(beta == 0.0))

    idxs = []
    for b in range(B):
        ix = np.nonzero(mask[b] != 0)[0]
        if len(ix) == 0:
            # all-masked row: the -1e8 bias is common to every key, so the
            # reference softmax reduces to plain softmax over all keys.
            ix = np.arange(S)
        idxs.append(ix)
    Lmax = max(len(ix) for ix in idxs)
    LPAD = max(2 * P, ((Lmax + P - 1) // P) * P)
    KT = LPAD // P

    in_maps = []
    for c in range(NCORES):
        b, g = divmod(c, 2)
        ix = idxs[b]
        L = len(ix)
        kc = np.zeros((LPAD, DMODEL), np.float32)
        kc[:L] = key[b][ix]
        vc = np.zeros((LPAD, DMODEL), np.float32)
        vc[:L] = value[b][ix]
        # -4.5 recenters exp so fp8 e2 stays below e4m3 max 240 (scores
        # reach ~9.3, exp overflows past 4.5+ln(240)=9.98); the shift is
        # uniform over keys so it cancels in normalization
        mb = np.full((LPAD,), -1e30, np.float32)
        mb[:L] = -4.5
        qrows = query[b, g * SQ:(g + 1) * SQ]
        in_maps.append({
            "qT": f8(qrows.T),
            "kT": f8(kc.T),
            "vT": f8(vc.T),
            "resid": np.ascontiguousarray(
                qrows.astype(ml_dtypes.bfloat16)),
            "wqT": wqT, "wkT": wkT, "wvT": wvT, "woT": woT,
            "maskb": np.ascontiguousarray(mb.reshape(KT, P).T),
            "gamma": gamma, "beta": beta,
        })
    return in_maps, LPAD, ln_trivial


def gather_out(results):
    out = np.empty((B, S, DMODEL), np.float32)
    for c in range(NCORES):
        b, g = divmod(c, 2)
        out[b, g * SQ:(g + 1) * SQ] = results[c]["out"]
    return out


def kernel(query, key, value, mask, Wq, Wk, Wv, Wo, ln_gamma, ln_beta):
    in_maps, LPAD, ln_trivial = make_in_maps(
        query, key, value, mask, Wq, Wk, Wv, Wo, ln_gamma, ln_beta)
    nc = _get_nc(LPAD, ln_trivial)
    res = run_bass_kernel_spmd(nc, in_maps, list(range(NCORES)))
    return gather_out(res.results)


# revision 22
# speedup vs baseline: 1.5256x; 1.0079x over previous
"""Trainium2 Bass kernel for a post-LN multi-head-attention block.

Reference computation (B=4, S=2048, D=1024, 16 heads x 64):
    q,k,v = x @ W{q,k,v}.T ; attn = softmax(q k^T/8 + mask) ; o = attn v
    out = LayerNorm(query + (o @ Wo.T)) * gamma + beta

Sharding: 8 cores = 4 batches x 2 query-halves (1024 query rows per core).
Each core computes all 16 heads for its query rows against the full
(mask-compacted) key set of its batch.  No collectives.

Key implementation choices:
  - keys with mask==0 contribute exactly 0 attention weight, so the host
    compacts key/value to the unmasked subset, padded to a multiple of 128
    (padding biased -1e30 so exp -> 0).
  - Q/K/V/O projections and attnV run in fp8e4m3 with DoubleRow perf
    mode (2 contraction rows per PE pass = 2x f32r/bf16 matmul
    throughput).  The attention output is only ~5% of the residual
    magnitude entering LayerNorm, so fp8 noise dilutes ~20x in the final
    output (measured end-to-end rel err ~9e-3 vs the 2e-2 budget).
  - scores keep fp16 operands; exp bias includes -4.5 so fp8
    e2 = exp(qk/8 - 4.5) stays below e4m3 max 240.
  - scores are computed transposed, scoresT[k, q], so softmax's
    k-reduction becomes a matmul reduction: V is augmented with a
    ones-column and attnV produces [out^T ; rowsum] in one PSUM group.
    Even heads use a [V | 1] augmentation (accumulator rows 0-64), odd
    heads a [1 | V] augmentation placed at PSUM partitions 63-127, so
    the normalized output lands directly in aoT partitions 64-127
    without a partition-shift DMA.
  - ScalarE (softmax exp) is the bottleneck engine (~137us of exp at
    1 elem/cycle/lane); everything else is scheduled around keeping it
    saturated: the prologue only computes the two projection chunks the
    first exp needs (their DMAs spread over four DGE queues), all other
    projections are deadline-ordered fills inside the kt loops, each
    iteration's first scores matmul is pre-issued at the tail of the
    previous iteration, and LayerNorm rstd uses Ln+Exp (one activation
    table set with the softmax Exp -> no table reloads).
  - loop nest is qc-outer / head-pair-inner: projections fill the
    TensorE during qc0's attention, and the output projection +
    LayerNorm for qc0 overlaps qc1's attention; the qc1 epilogue
    applies LayerNorm on ScalarE (idle after the last exp).
  - gamma/beta are ones/zeros in this problem; make_in_maps verifies at
    runtime and compiles the gamma/beta application out (falling back to
    a gpsimd apply if they were ever nontrivial).
"""

import numpy as np
import ml_dtypes

import concourse.bacc as bacc
import concourse.tile as tile
import concourse.bass as bass
from concourse import mybir
from concourse.bass_utils import run_bass_kernel_spmd

DMODEL = 1024
NHEAD = 16
HD = 64
B = 4
S = 2048
NCORES = 8
SQ = 1024          # query rows per core
P = 128
F32 = mybir.dt.float32
F32R = mybir.dt.float32r
F16 = mybir.dt.float16
BF16 = mybir.dt.bfloat16
F8 = mybir.dt.float8e4
DR = mybir.MatmulPerfMode.DoubleRow
NPF8 = ml_dtypes.float8_e4m3
ET = DMODEL // P   # 8 e-tiles (feature tiles)
DTL = DMODEL // P  # 8 d-tiles (contraction tiles)
NQC = SQ // 512    # 2 query chunks of 512
NEC = DMODEL // 512  # 2 feature chunks of 512
NPAIR = NHEAD // 2   # 8 head pairs; pair j = heads (2j, 2j+1) in e-tile j
AF = mybir.ActivationFunctionType


def _balanced_chunks(total, maxw=512):
    """Split `total` (a multiple of 128) into maximal chunks <= maxw,
    widest first: 512B-contiguous DMA runs dodge the sub-512B 2x
    latency penalty."""
    out, lo = [], 0
    while lo < total:
        w = min(maxw, total - lo)
        out.append((lo, lo + w))
        lo += w
    return out


def _build(LPAD, ln_trivial=True, do_compile=True, reps=1, phases=5):
    KT = LPAD // P
    KCH = _balanced_chunks(LPAD)
    QCH = _balanced_chunks(SQ)
    nc = bacc.Bacc("TRN2", target_bir_lowering=False, debug=False,
                   num_devices=NCORES)

    qT = nc.declare_dram_parameter("qT", [DMODEL, SQ], F8, isOutput=False)
    kT = nc.declare_dram_parameter("kT", [DMODEL, LPAD], F8, isOutput=False)
    vT = nc.declare_dram_parameter("vT", [DMODEL, LPAD], F8, isOutput=False)
    resid = nc.declare_dram_parameter("resid", [SQ, DMODEL], BF16,
                                      isOutput=False)
    wqT = nc.declare_dram_parameter("wqT", [DMODEL, DMODEL], F8, isOutput=False)
    wkT = nc.declare_dram_parameter("wkT", [DMODEL, DMODEL], F8, isOutput=False)
    wvT = nc.declare_dram_parameter("wvT", [DMODEL, DMODEL], F8, isOutput=False)
    woT = nc.declare_dram_parameter("woT", [DMODEL, DMODEL], F8, isOutput=False)
    maskb = nc.declare_dram_parameter("maskb", [P, KT], F32, isOutput=False)
    gamma = nc.declare_dram_parameter("gamma", [DMODEL], F32, isOutput=False)
    beta = nc.declare_dram_parameter("beta", [DMODEL], F32, isOutput=False)
    out = nc.declare_dram_parameter("out", [SQ, DMODEL], F32, isOutput=True)

    def dram3(ap):
        # (o*P, width) DRAM tensor viewed as [p, o, width]
        return ap.rearrange("(o p) w -> p o w", p=P)

    with tile.TileContext(nc) as tc:
        with (
            tc.tile_pool(name="keep", bufs=1) as keep,      # long-lived SBUF
            tc.tile_pool(name="dpool", bufs=2) as dpool,
            tc.tile_pool(name="pproj", bufs=2, space="PSUM") as pproj,
            tc.tile_pool(name="pscore", bufs=2, space="PSUM") as pscore,
            tc.tile_pool(name="pattn", bufs=1, space="PSUM") as pattn,
        ):
            for _rep in range(reps):
                self_body(nc, tc, keep, dpool, pproj, pscore, pattn, phases,
                          ln_trivial, LPAD, KT, KCH, QCH,
                          qT, kT, vT, resid, wqT, wkT, wvT, woT,
                          maskb, gamma, beta, out, dram3)
    if do_compile:
        nc.compile()
    return nc


def self_body(nc, tc, keep, dpool, pproj, pscore, pattn, phases, ln_trivial,
              LPAD, KT, KCH, QCH,
              qT, kT, vT, resid, wqT, wkT, wvT, woT, maskb, gamma, beta,
              out, dram3):
    NKTP = KT // 2            # full DoubleRow kt-pairs in attnV
    ODD = KT % 2
    NKC = len(KCH)
    NQCH = len(QCH)

    # ---- long-lived tensors ----
    vaug = keep.tile([P, KT, NHEAD, HD + 1], F8)     # [k | head | V,1]
    aoT = keep.tile([P, ET, SQ], F8)                 # attn out^T (d' on part)
    maskb_sb = keep.tile([P, KT], F32)
    nc.sync.dma_start(out=maskb_sb, in_=maskb.ap())
    ones_sb = keep.tile([65, 128], F32R)
    nc.vector.memset(ones_sb[64:65, :].bitcast(F32), 1.0)
    nc.vector.memset(ones_sb[0:1, :].bitcast(F32), 1.0)
    nc.vector.memset(vaug[:, :, :, HD:HD + 1], 1.0)
    kts = [keep.tile([P, LPAD], F16, name=f"kts{j}")
           for j in range(NPAIR)]
    qts = [keep.tile([P, SQ], F16, name=f"qts{j}")
           for j in range(NPAIR)]

    vT3 = dram3(vT.ap())
    wvT3 = dram3(wvT.ap())
    kT3 = dram3(kT.ap())
    qT3 = dram3(qT.ap())
    wkT3 = dram3(wkT.ap())
    wqT3 = dram3(wqT.ap())
    wk_sb = keep.tile([P, DTL, DMODEL], F8)
    wq_sb = keep.tile([P, DTL, DMODEL], F8)
    wv_sb = keep.tile([P, DTL, DMODEL], F8)
    kin = keep.tile([P, DTL, LPAD], F8)
    qin = keep.tile([P, DTL, SQ], F8)
    vin = keep.tile([P, DTL, LPAD], F8)
    K0 = KCH[0][1]
    Q0 = QCH[0][1]
    PKC0 = K0
    W0 = 512               # weight slice covering pairs 0-3
    # DMA bandwidth is one shared resource, so queue parallelism cannot
    # help the ramp -- ordering can.  Everything goes on the SP queue in
    # strict first-use order: the four transfers the first exp needs
    # (pair-0 weight columns, first key/query chunks) lead the stream.
    # DMA bandwidth is one shared resource, so the stream is ordered by
    # first use.  NARROW_DMA additionally front-loads 128-col weight
    # slices (sub-512B contiguous runs pay a penalty, but they unblock
    # pair 0/1 sooner).
    import os as _os
    if _os.environ.get("KERNEL_NARROW_DMA", "1") == "1":
        W1 = min(2 * P, DMODEL)
        nc.sync.dma_start(out=wk_sb[:, :, 0:P], in_=wkT3[:, :, 0:P])
        nc.sync.dma_start(out=kin[:, :, 0:K0], in_=kT3[:, :, 0:K0])
        nc.sync.dma_start(out=wq_sb[:, :, 0:P], in_=wqT3[:, :, 0:P])
        nc.sync.dma_start(out=qin[:, :, 0:Q0], in_=qT3[:, :, 0:Q0])
        if K0 < LPAD:
            nc.sync.dma_start(out=kin[:, :, K0:LPAD],
                              in_=kT3[:, :, K0:LPAD])
        nc.sync.dma_start(out=wk_sb[:, :, P:W1], in_=wkT3[:, :, P:W1])
        nc.sync.dma_start(out=wq_sb[:, :, P:W1], in_=wqT3[:, :, P:W1])
        nc.sync.dma_start(out=wv_sb[:, :, 0:P], in_=wvT3[:, :, 0:P])
        nc.sync.dma_start(out=vin[:, :, 0:K0], in_=vT3[:, :, 0:K0])
        if K0 < LPAD:
            nc.sync.dma_start(out=vin[:, :, K0:LPAD],
                              in_=vT3[:, :, K0:LPAD])
        nc.sync.dma_start(out=wv_sb[:, :, P:W1], in_=wvT3[:, :, P:W1])
        nc.sync.dma_start(out=wk_sb[:, :, W1:DMODEL],
                          in_=wkT3[:, :, W1:DMODEL])
        nc.sync.dma_start(out=wq_sb[:, :, W1:DMODEL],
                          in_=wqT3[:, :, W1:DMODEL])
        nc.sync.dma_start(out=wv_sb[:, :, W1:DMODEL],
                          in_=wvT3[:, :, W1:DMODEL])
    else:
        W0_ = 512
        nc.sync.dma_start(out=wk_sb[:, :, 0:W0_], in_=wkT3[:, :, 0:W0_])
        nc.sync.dma_start(out=kin[:, :, 0:K0], in_=kT3[:, :, 0:K0])
        nc.sync.dma_start(out=wq_sb[:, :, 0:W0_], in_=wqT3[:, :, 0:W0_])
        nc.sync.dma_start(out=qin[:, :, 0:Q0], in_=qT3[:, :, 0:Q0])
        if K0 < LPAD:
            nc.sync.dma_start(out=kin[:, :, K0:LPAD],
                              in_=kT3[:, :, K0:LPAD])
        nc.sync.dma_start(out=wv_sb[:, :, 0:W0_], in_=wvT3[:, :, 0:W0_])
        nc.sync.dma_start(out=vin[:, :, 0:K0], in_=vT3[:, :, 0:K0])
        if K0 < LPAD:
            nc.sync.dma_start(out=vin[:, :, K0:LPAD],
                              in_=vT3[:, :, K0:LPAD])
        nc.sync.dma_start(out=wk_sb[:, :, W0_:DMODEL],
                          in_=wkT3[:, :, W0_:DMODEL])
        nc.sync.dma_start(out=wq_sb[:, :, W0_:DMODEL],
                          in_=wqT3[:, :, W0_:DMODEL])
        nc.sync.dma_start(out=wv_sb[:, :, W0_:DMODEL],
                          in_=wvT3[:, :, W0_:DMODEL])
    for (lo, hi) in QCH[1:]:
        nc.sync.dma_start(out=qin[:, :, lo:hi], in_=qT3[:, :, lo:hi])

    def kth(j, lo, hi):
        """K^T projection thunk: kts[j][:, lo:hi] (fp8 DoubleRow, drained
        straight to fp16)."""
        def f(j=j, lo=lo, hi=hi):
            w = hi - lo
            ps = pproj.tile([P, 512], F32, tag="pp")
            for dt in range(DTL // 2):
                nc.tensor.matmul(
                    ps[:, :w],
                    lhsT=wk_sb[:, 2 * dt:2 * dt + 2, j * P:(j + 1) * P],
                    rhs=kin[:, 2 * dt:2 * dt + 2, lo:hi],
                    start=(dt == 0), stop=(dt == DTL // 2 - 1),
                    perf_mode=DR)
            nc.vector.tensor_copy(out=kts[j][:, lo:hi], in_=ps[:, :w])
        return f

    def qth(j, c):
        """Q^T projection thunk: qts[j][:, QCH[c]]."""
        def f(j=j, c=c):
            lo, hi = QCH[c]
            w = hi - lo
            ps = pproj.tile([P, 512], F32, tag="pp")
            for dt in range(DTL // 2):
                nc.tensor.matmul(
                    ps[:, :w],
                    lhsT=wq_sb[:, 2 * dt:2 * dt + 2, j * P:(j + 1) * P],
                    rhs=qin[:, 2 * dt:2 * dt + 2, lo:hi],
                    start=(dt == 0), stop=(dt == DTL // 2 - 1),
                    perf_mode=DR)
            nc.vector.tensor_copy(out=qts[j][:, lo:hi], in_=ps[:, :w])
        return f

    # V-projection per pair: kt-blocks grouped ~3 per thunk so a pair's
    # V work fits one fill slot
    _vstep = min((KT + 2) // 3, 4)
    VGRP = [(a, min(a + _vstep, KT)) for a in range(0, KT, _vstep)]
    NVG = len(VGRP)

    def vth(j, g):
        def f(j=j, g=g):
            g0, g1 = VGRP[g]
            gl = g1 - g0
            ps = pproj.tile([P, 512], F32, tag="pp")
            for i in range(gl):
                for dt in range(DTL // 2):
                    nc.tensor.matmul(
                        ps[:, i * P:(i + 1) * P],
                        lhsT=vin[:, 2 * dt:2 * dt + 2,
                                 (g0 + i) * P:(g0 + i + 1) * P],
                        rhs=wv_sb[:, 2 * dt:2 * dt + 2,
                                  j * P:(j + 1) * P],
                        start=(dt == 0), stop=(dt == DTL // 2 - 1),
                        perf_mode=DR)
            nc.vector.tensor_copy(
                out=vaug[:, g0:g1, 2 * j:2 * j + 2, 0:HD],
                in_=ps[:, 0:gl * P].rearrange(
                    "p (k h x) -> p k h x", h=2, x=HD))
        return f

    # ---- phase E pieces (output proj + residual + layernorm);
    # wo DMA is issued after the prologue so it doesn't delay vin ----
    wo_sb = keep.tile([P, DTL, DMODEL], F8)
    if not ln_trivial:
        gamma_sb = keep.tile([P, DMODEL], F32)
        beta_sb = keep.tile([P, DMODEL], F32)
    eps_sb = keep.tile([P, 1], F32)
    nc.vector.memset(eps_sb, 1e-5)
    resid3 = dram3(resid.ap())
    out3 = dram3(out.ap())

    NST = SQ // P
    mv_all = keep.tile([P, NST, 2], F32)     # per-block (mean, var)
    rstd_all = keep.tile([P, NST, 2], F32)   # (scratch ln, rstd)
    xts = {}

    def stE_pre(st):
        """Output projection + residual + bn stats for query block st."""
        rin = rin_pre[:, st, :]
        x_t = dpool.tile([P, DMODEL], F32, tag="x", bufs=5)
        xts[st] = x_t
        for ec in range(NEC):
            ps = pproj.tile([P, 512], F32, tag="pp")
            for dj in range(DTL // 2):
                nc.tensor.matmul(
                    ps,
                    lhsT=aoT[:, 2 * dj:2 * dj + 2, st * P:(st + 1) * P],
                    rhs=wo_sb[:, 2 * dj:2 * dj + 2,
                              ec * 512:(ec + 1) * 512],
                    start=(dj == 0), stop=(dj == DTL // 2 - 1),
                    perf_mode=DR)
            nc.vector.tensor_add(
                out=x_t[:, ec * 512:(ec + 1) * 512],
                in0=ps, in1=rin[:, ec * 512:(ec + 1) * 512])
        stats = dpool.tile([P, 2, 6], F32, tag="stats", bufs=2)
        nc.vector.bn_stats(out=stats[:, 0, :], in_=x_t[:, 0:512])
        nc.vector.bn_stats(out=stats[:, 1, :], in_=x_t[:, 512:1024])
        nc.vector.bn_aggr(out=mv_all[:, st, :], in_=stats)

    magic_sb = keep.tile([P, NST], mybir.dt.int32)
    nc.vector.memset(magic_sb, 0x5F3759DF)

    def stE_rstd(lo, hi):
        """rstd = 1/sqrt(var+eps) for query blocks [lo, hi) via the
        fast-inverse-square-root bit trick + 2 Newton steps, on DVE.
        Using ScalarE's Sqrt here would evict the softmax Exp activation
        table (~2.7us reload each way); the DVE ops are only a few
        elements per partition."""
        n = hi - lo
        t = rstd_all[:, lo:hi, 0]
        y = rstd_all[:, lo:hi, 1]
        a_t = dpool.tile([P, NST], F32, tag="nwt", bufs=2)
        a = a_t[:, 0:n]
        eng = nc.vector
        eng.tensor_scalar_add(t, mv_all[:, lo:hi, 1], 1e-5)
        eng.tensor_scalar(y.bitcast(mybir.dt.int32), t.bitcast(
            mybir.dt.int32), 1, None, op0=mybir.AluOpType.arith_shift_right)
        eng.tensor_tensor(y.bitcast(mybir.dt.int32), magic_sb[:, 0:n],
                          y.bitcast(mybir.dt.int32),
                          mybir.AluOpType.subtract)
        for _ in range(2):
            eng.tensor_tensor(a, y, y, mybir.AluOpType.mult)
            eng.tensor_tensor(a, a, t, mybir.AluOpType.mult)
            eng.tensor_scalar(a, a, -0.5, 1.5, op0=mybir.AluOpType.mult,
                              op1=mybir.AluOpType.add)
            eng.tensor_tensor(y, y, a, mybir.AluOpType.mult)

    def stE_post(st, on_act=False, dma_eng=None):
        """LayerNorm application + store for query block st.  The
        normalize runs on DVE mid-stream (ScalarE is exp-bound there)
        but on ScalarE for the tail blocks, where it idles and DVE is
        the pole."""
        x_t = xts.pop(st)
        y_t = dpool.tile([P, DMODEL], F32, tag="y", bufs=2)
        if on_act:
            nb = dpool.tile([P, 1], F32, tag="nb", bufs=2)
            nc.vector.tensor_tensor(nb, mv_all[:, st, 0:1],
                                    rstd_all[:, st, 1:2],
                                    mybir.AluOpType.mult)
            nc.vector.tensor_scalar_mul(nb, nb, -1.0)
            nc.scalar.activation(
                out=y_t, in_=x_t,
                func=AF.Identity,
                scale=rstd_all[:, st, 1:2], bias=nb[:, 0:1])
        else:
            nc.vector.tensor_scalar(
                out=y_t, in0=x_t, scalar1=mv_all[:, st, 0:1],
                scalar2=rstd_all[:, st, 1:2],
                op0=mybir.AluOpType.subtract, op1=mybir.AluOpType.mult)
        if not ln_trivial:
            nc.gpsimd.tensor_mul(out=y_t, in0=y_t, in1=gamma_sb)
            nc.gpsimd.tensor_add(out=y_t, in0=y_t, in1=beta_sb)
        (dma_eng or nc.sync).dma_start(out=out3[:, st, :], in_=y_t)

    # ======== prologue: just the two chunks the first exp needs ========
    kth(0, 0, PKC0)()
    qth(0, 0)()
    nc.sync.dma_start(out=wo_sb, in_=dram3(woT.ap()))
    if not ln_trivial:
        nc.gpsimd.dma_start(out=gamma_sb, in_=bass.AP(
            tensor=gamma.ap().tensor, offset=0, ap=[[0, P], [1, DMODEL]]))
        nc.gpsimd.dma_start(out=beta_sb, in_=bass.AP(
            tensor=beta.ap().tensor, offset=0, ap=[[0, P], [1, DMODEL]]))
    rin_pre = keep.tile([P, SQ // P, DMODEL], BF16)
    for st in range(SQ // P):
        nc.sync.dma_start(out=rin_pre[:, st, :], in_=resid3[:, st, :])

    if phases < 4:
        for j in range(NPAIR):
            if PKC0 < K0:
                kth(0, PKC0, K0)() if j == 0 else None
            for c in range(NKC):
                if (j, c) != (0, 0):
                    kth(j, *KCH[c])()
            if j != 0:
                kth(j, 0, PKC0)()
            for c in range(NQCH):
                if (j, c) != (0, 0):
                    qth(j, c)()
            for g in range(NVG):
                vth(j, g)()
        nc.sync.dma_start(out=out.ap()[0:P, 0:LPAD // 2],
                          in_=kts[NPAIR - 1].bitcast(F32))
        nc.sync.dma_start(out=out.ap()[P:2 * P, 0:SQ // 2],
                          in_=qts[NPAIR - 1].bitcast(F32))
        return

    # ---- fill schedule (qc0): everything except the prologue's two
    # thunks, deadline-ordered into the kt slots of each pair iteration.
    # Deadlines (slot = kt index within iteration j):
    #   vth(j,0) before attnv(0) at kt=3; kth(j,c>=2) before scores(8);
    #   vth(j,1)/(j,2) before later attnv; kth(j+1,0)/qth(j+1,0)/
    #   kth(j+1,1) before iteration j+1 needs them; qth(*,1) before qc1.
    fills_qc0 = []
    for j in range(NPAIR):
        kches = []
        if j == 0 and PKC0 < K0:
            kches.append((PKC0, K0))
        kches += [KCH[c] for c in range(1, NKC)]
        vgs = list(range(NVG))
        fl = []
        while kches or vgs:
            if kches:
                fl.append(kth(j, *kches.pop(0)))
            if vgs:
                fl.append(vth(j, vgs.pop(0)))
        if j + 1 < NPAIR:
            fl += [kth(j + 1, 0, K0), qth(j + 1, 0)]
        if j > 0:
            fl += [qth(j - 1, c) for c in range(1, NQCH)]
        if j == NPAIR - 1:
            fl += [qth(j, c) for c in range(1, NQCH)]
        fills_qc0.append(fl)

    # ======== main: qc outer, head-pair inner ========
    # qc0: projections as TensorE filler.  qc1: phase E for qc0's rows as
    # filler.  Epilogues deferred one iteration so ScalarE never waits at
    # iteration boundaries; each iteration's scores(0) is pre-issued at
    # the tail of the previous iteration for the same reason.
    def make_iter(qc, j, last=False):
        """One (query-chunk, head-pair) attention iteration.

        `last`: the final iteration's odd head runs a V-only matmul into
        PSUM partitions 64-127 (base 64) plus a rowsum matmul from the
        ones column (base 32), so its epilogue writes aoT[64:128]
        directly instead of shifting partitions with a DMA — that DMA
        latency would sit on the serial tail."""
        qsl = slice(qc * 512, (qc + 1) * 512)
        avA = pattn.tile([HD + 1, 512], F32, tag="avA")
        avB_t = pattn.tile([P, 512], F32, tag="avB")
        es = {}

        def scores(kt):
            ps = pscore.tile([P, 1024], F32, tag="ps_s")
            nc.tensor.matmul(
                ps[:, 0:512],
                lhsT=kts[j][0:64, kt * P:(kt + 1) * P],
                rhs=qts[j][0:64, qsl], start=True, stop=True)
            nc.tensor.matmul(
                ps[:, 512:1024],
                lhsT=kts[j][64:128, kt * P:(kt + 1) * P],
                rhs=qts[j][64:128, qsl], start=True, stop=True)
            ktp, par = divmod(kt, 2)
            if par == 0:
                e2 = dpool.tile([P, 2, 2, 512], F8, tag="e2", bufs=4)
                es[ktp] = e2
            else:
                e2 = es[ktp]
            nc.scalar.activation(
                out=e2[:, par, :, :],
                in_=ps.rearrange("p (a b) -> p a b", b=512),
                func=AF.Exp,
                bias=maskb_sb[:, kt:kt + 1], scale=0.125)

        hA = 2 * j
        hB = 2 * j + 1

        def attnv(ktp):
            st = (ktp == 0)
            sp = (ktp == NKTP - 1) and not ODD
            if ktp < NKTP:
                e2 = es[ktp]
                vsl = slice(2 * ktp, 2 * ktp + 2)
                nc.tensor.matmul(
                    avA, lhsT=vaug[:, vsl, hA, :],
                    rhs=e2[:, :, 0, :],
                    start=st, stop=sp, perf_mode=DR)
                if not last:
                    nc.tensor.matmul(
                        avB_t[0:HD + 1, :], lhsT=vaug[:, vsl, hB, :],
                        rhs=e2[:, :, 1, :],
                        start=st, stop=sp, perf_mode=DR)
                else:
                    # DoubleRow can't target base partition 64 (column
                    # tiling and DR are mutually exclusive), so the
                    # direct-placement passes run one kt at a time
                    for par in range(2):
                        kt = 2 * ktp + par
                        nc.tensor.matmul(
                            avB_t[64:128, :],
                            lhsT=vaug[:, kt, hB, 0:HD],
                            rhs=e2[:, par, 1, :],
                            start=(st and par == 0),
                            stop=(sp and par == 1))
                        nc.tensor.matmul(
                            avB_t[0:1, :],
                            lhsT=vaug[:, kt, hB, HD:HD + 1],
                            rhs=e2[:, par, 1, :],
                            start=(st and par == 0),
                            stop=(sp and par == 1))
            else:          # leftover single-kt pass (odd KT)
                e2 = es[ktp]
                nc.tensor.matmul(
                    avA, lhsT=vaug[:, KT - 1, hA, :],
                    rhs=e2[:, 0, 0, :],
                    start=False, stop=True)
                if not last:
                    nc.tensor.matmul(
                        avB_t[0:HD + 1, :], lhsT=vaug[:, KT - 1, hB, :],
                        rhs=e2[:, 0, 1, :],
                        start=False, stop=True)
                else:
                    nc.tensor.matmul(
                        avB_t[64:128, :],
                        lhsT=vaug[:, KT - 1, hB, 0:HD],
                        rhs=e2[:, 0, 1, :],
                        start=False, stop=True)
                    nc.tensor.matmul(
                        avB_t[0:1, :],
                        lhsT=vaug[:, KT - 1, hB, HD:HD + 1],
                        rhs=e2[:, 0, 1, :],
                        start=False, stop=True)

        def epilogue():
            # deferred pipeline tail
            if NKTP > 0:
                attnv(NKTP - 1)
            if ODD:
                attnv(NKTP)
            # normalize in place: aoT = av / rowsum, reading the PSUM
            # accumulators directly.  1/rowsum is broadcast across
            # partitions with a K=1 matmul; DVE reads at most one PSUM
            # operand, so the broadcast drains to SBUF first.
            # DVE lanes are hardwired to partitions: reciprocal outputs
            # stay on the rowsum's own partition (64 regular, 32 last)
            rrow = 0 if last else 64
            rsB = avB_t[rrow:rrow + 1, :]
            rc = dpool.tile([P, 1024], F32R, tag="recip", bufs=2)
            with nc.allow_low_precision(
                    reason="f32r recip feeds f32r matmul"):
                nc.vector.reciprocal(
                    out=rc[64:65, 0:512], in_=avA[64:65, :])
                nc.vector.reciprocal(
                    out=rc[rrow:rrow + 1, 512:1024], in_=rsB)
            rbA = pproj.tile([P, 512], F32, tag="pp")
            rbB = pproj.tile([P, 512], F32, tag="pp")
            bsl = slice(64, 128) if last else slice(0, 64)
            nc.tensor.matmul(rbA[0:64, :], lhsT=ones_sb[64:65, 0:64],
                             rhs=rc[64:65, 0:512],
                             start=True, stop=True)
            # `last` broadcasts to all 128 partitions (tile_position
            # (0,0)); a direct base-64 K=1 matmul fails the ISA check
            nc.tensor.matmul(rbB[0:P, :] if last else rbB[0:64, :],
                             lhsT=(ones_sb[0:1, 0:P] if last
                                   else ones_sb[64:65, 0:64]),
                             rhs=rc[rrow:rrow + 1, 512:1024],
                             start=True, stop=True)
            nc.vector.tensor_copy(out=rc[0:64, 0:512], in_=rbA[0:64, :])
            nc.vector.tensor_copy(out=rc[bsl, 512:1024], in_=rbB[bsl, :])
            nc.vector.tensor_tensor(
                aoT[0:64, j, qsl], avA[0:64, :],
                rc[0:64, 0:512].bitcast(F32),
                mybir.AluOpType.mult)
            if last:
                nc.vector.tensor_tensor(
                    aoT[64:128, j, qsl], avB_t[64:128, :],
                    rc[64:128, 512:1024].bitcast(F32),
                    mybir.AluOpType.mult)
            else:
                av_f8 = dpool.tile([64, 512], F8, tag="av_f8", bufs=2)
                nc.vector.tensor_tensor(
                    av_f8, avB_t[0:64, :],
                    rc[0:64, 512:1024].bitcast(F32),
                    mybir.AluOpType.mult)
                nc.sync.dma_start(out=aoT[64:128, j, qsl], in_=av_f8)

        return scores, attnv, epilogue

    iters = [(qc, j) for qc in range(NQC) for j in range(NPAIR)]
    pend = [None]
    cur = make_iter(*iters[0], last=(len(iters) == 1))
    cur[0](0)                      # scores(0) of the first iteration
    stqueue = []
    for idx, (qc, j) in enumerate(iters):
        scores, attnv, epilogue = cur
        if qc == 1 and j == 0:
            # phase E for qc0's blocks, one piece per head-pair slot
            stqueue = [lambda s=s: stE_pre(s) for s in range(NST // 2)]
            stqueue.append(lambda: (stE_rstd(0, NST // 2), stE_post(0)))
            stqueue += [lambda s=s: stE_post(s) for s in range(1, NST // 2)]
        if KT <= 2 and pend[0] is not None:
            pend[0]()
            pend[0] = None
        fill = list(fills_qc0[j]) if qc == 0 else []
        stfill = [stqueue.pop(0)] if (qc == 1 and stqueue) else []
        for kt in range(1, KT):
            scores(kt)
            if kt == min(3, KT - 1) and pend[0] is not None:
                # prev iteration's attnv tail + epilogue: issued behind
                # three scores so its PE-side wait on the DVE reciprocal
                # never starves ScalarE at the boundary
                pend[0]()
                pend[0] = None
            if kt % 2 == 1 and kt >= 3:
                attnv(kt // 2 - 1)
            if fill:
                fill.pop(0)()
            elif stfill and kt >= 4:
                stfill.pop(0)()
        # pre-issue the next iteration's first scores+exp so ScalarE
        # never waits at the boundary
        if idx + 1 < len(iters):
            nxt = make_iter(*iters[idx + 1], last=(idx + 2 == len(iters)))
            nxt[0](0)
        else:
            nxt = None
        while fill:
            fill.pop(0)()
        if nxt is None:
            epilogue()         # last iteration: run now, don't defer
        else:
            pend[0] = epilogue
        cur = nxt

    if pend[0] is not None:
        pend[0]()
        pend[0] = None

    if phases < 5:
        return

    # phase E tail: qc1's query blocks (qc0's ran as qc1 attention
    # fills).  ScalarE is idle after the last exp, so the normalize runs
    # there (on_act) while DVE works the adds/stats chain; out DMAs
    # alternate queues so the stores overlap.
    h = NST // 2
    stE_pre(h)
    stE_pre(h + 1)
    stE_rstd(h, h + 1)
    stE_post(h, on_act=True)
    stE_pre(h + 2)
    stE_rstd(h + 1, h + 2)
    stE_post(h + 1, on_act=True)
    stE_pre(h + 3)
    stE_rstd(h + 2, h + 3)
    stE_post(h + 2, on_act=True)
    stE_rstd(h + 3, h + 4)
    stE_post(h + 3, on_act=True)


_cache = {}


def _get_nc(LPAD, ln_trivial):
    key = (LPAD, ln_trivial)
    if key not in _cache:
        _cache[key] = _build(LPAD, ln_trivial)
    return _cache[key]


def make_in_maps(query, key, value, mask, Wq, Wk, Wv, Wo, ln_gamma, ln_beta):
    """Host-side sharding: returns (in_maps, LPAD, ln_trivial)."""
    f = lambda a: np.ascontiguousarray(np.asarray(a, np.float32))
    f8 = lambda a: np.ascontiguousarray(
        np.asarray(np.asarray(a, np.float32), NPF8))
    query, key, value = f(query), f(key), f(value)
    mask = np.asarray(mask)
    wqT = f8(np.asarray(Wq, np.float32).T)
    wkT = f8(np.asarray(Wk, np.float32).T)
    wvT = f8(np.asarray(Wv, np.float32).T)
    woT = f8(np.asarray(Wo, np.float32).T)
    gamma, beta = f(ln_gamma), f(ln_beta)
    ln_trivial = bool(np.all(gamma == 1.0) and np.all(beta == 0.0))

    idxs = []
    for b in range(B):
        ix = np.nonzero(mask[b] != 0)[0]
        if len(ix) == 0:
            # all-masked row: the -1e8 bias is common to every key, so the
            # reference softmax reduces to plain softmax over all keys.
            ix = np.arange(S)
        idxs.append(ix)
    Lmax = max(len(ix) for ix in idxs)
    LPAD = max(2 * P, ((Lmax + P - 1) // P) * P)
    KT = LPAD // P

    in_maps = []
    for c in range(NCORES):
        b, g = divmod(c, 2)
        ix = idxs[b]
        L = len(ix)
        kc = np.zeros((LPAD, DMODEL), np.float32)
        kc[:L] = key[b][ix]
        vc = np.zeros((LPAD, DMODEL), np.float32)
        vc[:L] = value[b][ix]
        # -4.5 recenters exp so fp8 e2 stays below e4m3 max 240 (scores
        # reach ~9.3, exp overflows past 4.5+ln(240)=9.98); the shift is
        # uniform over keys so it cancels in normalization
        mb = np.full((LPAD,), -1e30, np.float32)
        mb[:L] = -4.5
        qrows = query[b, g * SQ:(g + 1) * SQ]
        in_maps.append({
            "qT": f8(qrows.T),
            "kT": f8(kc.T),
            "vT": f8(vc.T),
            "resid": np.ascontiguousarray(
                qrows.astype(ml_dtypes.bfloat16)),
            "wqT": wqT, "wkT": wkT, "wvT": wvT, "woT": woT,
            "maskb": np.ascontiguousarray(mb.reshape(KT, P).T),
            "gamma": gamma, "beta": beta,
        })
    return in_maps, LPAD, ln_trivial


def gather_out(results):
    out = np.empty((B, S, DMODEL), np.float32)
    for c in range(NCORES):
        b, g = divmod(c, 2)
        out[b, g * SQ:(g + 1) * SQ] = results[c]["out"]
    return out


def kernel(query, key, value, mask, Wq, Wk, Wv, Wo, ln_gamma, ln_beta):
    in_maps, LPAD, ln_trivial = make_in_maps(
        query, key, value, mask, Wq, Wk, Wv, Wo, ln_gamma, ln_beta)
    nc = _get_nc(LPAD, ln_trivial)
    res = run_bass_kernel_spmd(nc, in_maps, list(range(NCORES)))
    return gather_out(res.results)
